# revision 1
# baseline (speedup 1.0000x reference)
"""Trainium2 Bass kernel for nn_AbsoluteRelativePositionEmbedding.

Two SPMD launches over 8 NeuronCores:
  Launch A (8 cores): each core handles HALF of one batch's query rows.
    kNN(16) over all N=8192 candidates -> gather neighbor coords ->
    conv1(6->32) -> x1 + per-channel partial sums (for GroupNorm-1 stats).
  Host: merges the tiny GN1 partial sums per batch -> scale/shift vectors.
  Launch B (8 cores, batch-parallel): gn1+ELU -> conv2(32->64) -> gn2+ELU
    (stats on device) -> max over K -> conv3 -> gn3+ELU -> conv4 -> gn4+ELU.

The distance matmul uses a 3-level bf16 split (21 contraction rows) which
matches fp32 accuracy at 4x the PE rate. ELU(x)+1 = min(exp(x),1)+relu(x)
is used internally; the +1 is folded into the next conv's bias.
"""
import sys
sys.path.insert(0, '/opt/trn_rl_repo')

import numpy as np
import ml_dtypes

import concourse.bass as bass
import concourse.bacc as bacc
import concourse.mybir as mybir
import concourse.tile as tile
from concourse.bass import IndirectOffsetOnAxis
from concourse.bass_utils import run_bass_kernel_spmd
from concourse.masks import make_identity

F32 = mybir.dt.float32
U32 = mybir.dt.uint32
BF = mybir.dt.bfloat16
AF = mybir.ActivationFunctionType
ALU = mybir.AluOpType
AX = mybir.AxisListType

K = 16
EPS = 1e-5
NEG = -1e30


def _elu1_affine_inplace(nc, pool, x, s, t):
    """x = elu(s*x + t) + 1 in place, using shared [128, f] scratch tags."""
    p, f = x.shape[0], x.shape[1]
    m = pool.tile([128, f], F32, tag="elu_scr_m")
    e = pool.tile([128, f], F32, tag="elu_scr_e")
    nc.scalar.activation(m[0:p, :], x, AF.Relu, scale=s, bias=t)
    nc.scalar.activation(e[0:p, :], x, AF.Exp, scale=s, bias=t)
    nc.vector.scalar_tensor_tensor(x, e[0:p, :], 1.0, m[0:p, :], ALU.min,
                                   ALU.add)


def _gn_coeffs(tc, nc, stats_pool, C, G, mean_c, ex2_c, bvec, gvec, tvec,
               indT, bindT, name, eps_ap):
    """Per-channel GN scale s / shift t so that gn(y+b) = s*y + t."""
    psp = tc.alloc_tile_pool(name=f"ps_{name}", bufs=1, space="PSUM")
    m1 = stats_pool.tile([C, 1], F32, tag=f"m1_{name}")
    m2 = stats_pool.tile([C, 1], F32, tag=f"m2_{name}")
    nc.vector.tensor_add(m1, mean_c, bvec)
    tmp = stats_pool.tile([C, 1], F32, tag=f"tmp_{name}")
    nc.vector.tensor_add(tmp, mean_c, m1)
    nc.vector.tensor_mul(tmp, tmp, bvec)
    nc.vector.tensor_add(m2, ex2_c, tmp)
    gm_ps = psp.tile([G, 1], F32, tag="gm_ps")
    gm_sb = stats_pool.tile([G, 1], F32, tag=f"gm_sb_{name}")
    bc1 = stats_pool.tile([C, 1], F32, tag=f"bc1_{name}")
    bc2 = stats_pool.tile([C, 1], F32, tag=f"bc2_{name}")
    bc_ps = psp.tile([C, 1], F32, tag="bc_ps")
    nc.tensor.matmul(gm_ps, indT, m1, start=True, stop=True)
    nc.scalar.copy(gm_sb, gm_ps)
    nc.tensor.matmul(bc_ps, bindT, gm_sb, start=True, stop=True)
    nc.scalar.copy(bc1, bc_ps)
    nc.tensor.matmul(gm_ps, indT, m2, start=True, stop=True)
    nc.scalar.copy(gm_sb, gm_ps)
    nc.tensor.matmul(bc_ps, bindT, gm_sb, start=True, stop=True)
    nc.scalar.copy(bc2, bc_ps)
    var = stats_pool.tile([C, 1], F32, tag=f"var_{name}")
    nc.vector.tensor_mul(tmp, bc1, bc1)
    nc.vector.tensor_sub(var, bc2, tmp)
    sd = stats_pool.tile([C, 1], F32, tag=f"sd_{name}")
    nc.scalar.activation(sd, var, AF.Sqrt, bias=eps_ap[0:C, :])
    s = stats_pool.tile([C, 1], F32, tag=f"s_{name}")
    nc.vector.reciprocal(s, sd)
    nc.vector.tensor_mul(s, s, gvec)
    t = stats_pool.tile([C, 1], F32, tag=f"t_{name}")
    nc.vector.tensor_sub(tmp, bc1, bvec)
    nc.vector.tensor_mul(tmp, s, tmp)
    nc.vector.tensor_sub(t, tvec, tmp)
    psp.release()
    return s, t


def build_phaseA(tc, outs, ins, N, R, ch=256):
    """kNN + gather + conv1 for R query rows against N candidates."""
    nc = tc.nc
    NTh = R // 128
    NCH = N // ch

    tbl_d = ins["tbl"]
    x1o_d = outs["x1o"]

    consts = tc.alloc_tile_pool(name="consts", bufs=1)
    persist = tc.alloc_tile_pool(name="persist", bufs=1)

    w1bTg = consts.tile([68, 32 * 16], F32, tag="w1bTg")
    nc.sync.dma_start(w1bTg, ins["w1bTg"])
    ident = consts.tile([128, 128], F32, tag="ident")
    make_identity(nc, ident)

    Q = persist.tile([4, R], F32, tag="Q")
    Qb = persist.tile([21, R], BF, tag="Qb")
    Cb = persist.tile([21, N], BF, tag="Cb")
    nc.sync.dma_start(Q, ins["pts"])
    nc.sync.dma_start(Qb, ins["qb"])
    nc.sync.dma_start(Cb, ins["cb"])

    accx1 = persist.tile([32, NTh * 4], F32, tag="accx1")
    accx1sq = persist.tile([32, NTh * 4], F32, tag="accx1sq")

    with tc.tile_pool(name="pa_sb", bufs=2) as pa, \
         tc.tile_pool(name="pa_nd", bufs=2) as pa_nd, \
         tc.tile_pool(name="pa_ps", bufs=2, space="PSUM") as psa, \
         tc.tile_pool(name="pa_ps1", bufs=2, space="PSUM") as psa1, \
         tc.tile_pool(name="pa_pst", bufs=2, space="PSUM") as psat:
        for t in range(NTh):
            r0 = t * 128
            ndsb = pa_nd.tile([128, N], F32, tag="ndsb")
            CW = min(1024, N)
            for cc in range(N // CW):
                nd_ps = psa.tile([128, CW], F32, tag="nd_ps")
                for h in range(CW // 512):
                    nc.tensor.matmul(
                        nd_ps[:, h * 512:(h + 1) * 512],
                        Qb[:, r0:r0 + 128],
                        Cb[:, cc * CW + h * 512:cc * CW + (h + 1) * 512],
                        start=True, stop=True)
                nc.scalar.copy(ndsb[:, cc * CW:(cc + 1) * CW], nd_ps)
            # top-8 per chunk
            cand = pa.tile([128, NCH * 8], F32, tag="cand")
            for cq in range(NCH):
                nc.vector.max(out=cand[:, cq * 8:(cq + 1) * 8],
                              in_=ndsb[:, cq * ch:(cq + 1) * ch])
            r1 = pa.tile([128, 8], F32, tag="r1")
            r2 = pa.tile([128, 8], F32, tag="r2")
            r3 = pa.tile([128, 8], F32, tag="r3")
            cand2 = pa.tile([128, NCH * 8], F32, tag="cand2")
            cand3 = pa.tile([128, NCH * 8], F32, tag="cand3")
            nc.vector.max(out=r1, in_=cand)
            nc.vector.match_replace(out=cand2, in_to_replace=r1,
                                    in_values=cand, imm_value=NEG)
            nc.vector.max(out=r2, in_=cand2)
            nc.vector.match_replace(out=cand3, in_to_replace=r2,
                                    in_values=cand2, imm_value=NEG)
            nc.vector.max(out=r3, in_=cand3)
            # winners: ranks 2..17 (rank 1 is self)
            wA = pa.tile([128, 8], F32, tag="wA")
            nc.vector.tensor_copy(wA[:, 0:7], r1[:, 1:8])
            nc.vector.tensor_copy(wA[:, 7:8], r3[:, 0:1])
            gidx = pa.tile([128, K], U32, tag="gidx")
            nc.vector.max_index(gidx[:, 0:8], wA, ndsb)
            nc.vector.max_index(gidx[:, 8:16], r2, ndsb)
            # gather neighbor coords (16B rows from the xyz0 table)
            gt = pa.tile([128, K * 4], F32, tag="gt")
            for k in range(K):
                nc.gpsimd.indirect_dma_start(
                    out=gt[:, 4 * k:4 * (k + 1)], out_offset=None,
                    in_=tbl_d,
                    in_offset=IndirectOffsetOnAxis(ap=gidx[:, k:k + 1],
                                                   axis=0))
            gtT_ps = psat.tile([64, 128], F32, tag="gtT_ps")
            nc.tensor.transpose(gtT_ps, gt, ident)
            rhs68 = pa.tile([68, 128], F32, tag="rhs68")
            nc.scalar.copy(rhs68[0:64, :], gtT_ps)
            nc.scalar.copy(rhs68[64:68, :], Q[:, r0:r0 + 128])
            # conv1 into (32, 128*K), position order (k, r)
            x1sb = pa.tile([32, 128 * K], F32, tag="x1sb")
            for q in range(K // 4):
                x1_ps = psa1.tile([32, 512], F32, tag="x1_ps")
                for j in range(4):
                    k = 4 * q + j
                    nc.tensor.matmul(x1_ps[:, j * 128:(j + 1) * 128],
                                     w1bTg[:, 32 * k:32 * (k + 1)], rhs68,
                                     start=True, stop=True)
                sl = slice(q * 512, (q + 1) * 512)
                nc.scalar.activation(
                    x1sb[:, sl], x1_ps, AF.Identity,
                    accum_out=accx1[:, 4 * t + q:4 * t + q + 1])
                sqsc = pa.tile([32, 512], F32, tag="sqsc")
                nc.scalar.activation(
                    sqsc, x1sb[:, sl], AF.Square,
                    accum_out=accx1sq[:, 4 * t + q:4 * t + q + 1])
            nc.sync.dma_start(x1o_d[t], x1sb)

    nc.sync.dma_start(outs["accx1o"], accx1)
    nc.sync.dma_start(outs["accx1sqo"], accx1sq)
    persist.release()
    consts.release()


def build_phaseBC(tc, outs, ins, N, ch=256):
    """gn1+ELU -> conv2 -> pool -> gn2 -> conv3/4 with on-device GN2-4."""
    nc = tc.nc
    NT = N // 128
    NK = N * K

    x1s_d = ins["x1s"]
    out_d = outs["out"]

    consts = tc.alloc_tile_pool(name="consts", bufs=1)
    stats_pool = tc.alloc_tile_pool(name="stats", bufs=1)
    persist = tc.alloc_tile_pool(name="persist", bufs=1)

    def load_const(name, shape):
        t = consts.tile(shape, F32, tag=name)
        nc.sync.dma_start(t, ins[name])
        return t

    w2T = load_const("w2T", [32, 64])
    w3T = load_const("w3T", [64, 128])
    w4T = load_const("w4T", [128, 256])
    vecs = {}
    for nm, c in (("b2", 64), ("g2", 64), ("t2", 64), ("b3", 128),
                  ("g3", 128), ("t3", 128)):
        vecs[nm] = load_const(nm, [c, 1])
    for nm in ("b4", "g4", "t4"):
        vecs[nm] = load_const(nm, [128, 2])
    s1 = load_const("s1", [32, 1])
    t1 = load_const("t1", [32, 1])

    def load_ind(Cc, Gg, name):
        return (load_const(f"indT_{name}", [Cc, Gg]),
                load_const(f"bindT_{name}", [Gg, Cc]))

    eps_t = consts.tile([128, 1], F32, tag="eps_t")
    nc.vector.memset(eps_t, EPS)
    ind64 = load_ind(64, 8, "c64")
    ind128g8 = load_ind(128, 8, "c128")
    ind128g4 = load_ind(128, 4, "c128h")

    accz = persist.tile([32, NT], F32, tag="accz")
    accysq = persist.tile([64, NT], F32, tag="accysq")
    ypool = persist.tile([64, N], F32, tag="ypool")

    # ============ PHASE B: gn1+elu + conv2 + pool ============
    with tc.tile_pool(name="pb_sb", bufs=2) as pb, \
         tc.tile_pool(name="pb_scr", bufs=1) as pbs, \
         tc.tile_pool(name="pb_ps", bufs=2, space="PSUM") as psb:
        for t in range(NT):
            r0 = t * 128
            x1sb = pb.tile([32, 128 * K], F32, tag="x1sb_b")
            nc.sync.dma_start(x1sb, x1s_d[t])
            m_ = pbs.tile([32, 128 * K], F32, tag="elu_m_b")
            e_ = pbs.tile([32, 128 * K], F32, tag="elu_e_b")
            nc.scalar.activation(m_, x1sb, AF.Relu, scale=s1, bias=t1)
            nc.scalar.activation(e_, x1sb, AF.Exp, scale=s1, bias=t1)
            z = pb.tile([32, 128 * K], F32, tag="z")
            # z = elu(gn1(x1)) + 1  (the +1 is folded into b2' on host)
            nc.vector.scalar_tensor_tensor(z, e_, 1.0, m_, ALU.min, ALU.add,
                                           accum_out=accz[:, t:t + 1])
            y_ps = psb.tile([64, 128 * K], F32, tag="y_ps")
            for h in range(4):
                nc.tensor.matmul(y_ps[:, h * 512:(h + 1) * 512],
                                 w2T, z[:, h * 512:(h + 1) * 512],
                                 start=True, stop=True)
            ysq = pbs.tile([64, 128 * K], F32, tag="ysq")
            nc.scalar.activation(ysq, y_ps, AF.Square,
                                 accum_out=accysq[:, t:t + 1])
            # max over K (position order (k, r): fold k halves)
            p1 = pbs.tile([64, 1024], F32, tag="p1")
            p2 = pbs.tile([64, 512], F32, tag="p2")
            p3 = pbs.tile([64, 256], F32, tag="p3")
            yh = pbs.tile([64, 1024], F32, tag="yh")
            nc.scalar.copy(yh, y_ps[:, 1024:2048])
            nc.vector.tensor_tensor(p1, y_ps[:, 0:1024], yh, op=ALU.max)
            nc.vector.tensor_tensor(p2, p1[:, 0:512], p1[:, 512:1024],
                                    op=ALU.max)
            nc.vector.tensor_tensor(p3, p2[:, 0:256], p2[:, 256:512],
                                    op=ALU.max)
            nc.vector.tensor_tensor(ypool[:, r0:r0 + 128], p3[:, 0:128],
                                    p3[:, 128:256], op=ALU.max)

    # ---- GN2 coefficients (mean via W2 @ sum(z)) ----
    zsum = stats_pool.tile([32, 1], F32, tag="zsum")
    nc.vector.tensor_reduce(zsum, accz, AX.X, ALU.add)
    gn2ps = tc.alloc_tile_pool(name="gn2ps", bufs=1, space="PSUM")
    ysum_ps = gn2ps.tile([64, 1], F32, tag="ysum_ps")
    nc.tensor.matmul(ysum_ps, w2T, zsum, start=True, stop=True)
    mean2 = stats_pool.tile([64, 1], F32, tag="mean2")
    ex22 = stats_pool.tile([64, 1], F32, tag="ex22")
    nc.scalar.copy(mean2, ysum_ps)
    gn2ps.release()
    nc.vector.tensor_reduce(ex22, accysq, AX.X, ALU.add)
    nc.vector.tensor_scalar_mul(mean2, mean2, 1.0 / NK)
    nc.vector.tensor_scalar_mul(ex22, ex22, 1.0 / NK)
    s2, t2 = _gn_coeffs(tc, nc, stats_pool, 64, 8, mean2, ex22,
                        vecs["b2"], vecs["g2"], vecs["t2"], *ind64,
                        name="gn2", eps_ap=eps_t)

    # ============ STAGE 3/4 on pooled (64, N) ============
    with tc.tile_pool(name="pc_sb", bufs=1) as pc, \
         tc.tile_pool(name="pc_ps", bufs=2, space="PSUM") as psc:
        u = pc.tile([64, N], F32, tag="u")
        nc.vector.tensor_copy(u, ypool)
        _elu1_affine_inplace(nc, pc, u, s2, t2)

        v0 = pc.tile([128, N], F32, tag="v0")
        sqscr = pc.tile([128, 512], F32, tag="sqscr")
        accv = stats_pool.tile([128, N // 512], F32, tag="accv")
        accvsq = stats_pool.tile([128, N // 512], F32, tag="accvsq")
        for cch in range(N // 512):
            sl = slice(cch * 512, (cch + 1) * 512)
            v_ps = psc.tile([128, 512], F32, tag="v_ps")
            nc.tensor.matmul(v_ps, w3T, u[:, sl], start=True, stop=True)
            nc.scalar.activation(v0[:, sl], v_ps, AF.Identity,
                                 accum_out=accv[:, cch:cch + 1])
            nc.scalar.activation(sqscr, v0[:, sl], AF.Square,
                                 accum_out=accvsq[:, cch:cch + 1])
        mean3 = stats_pool.tile([128, 1], F32, tag="mean3")
        ex23 = stats_pool.tile([128, 1], F32, tag="ex23")
        nc.vector.tensor_reduce(mean3, accv, AX.X, ALU.add)
        nc.vector.tensor_reduce(ex23, accvsq, AX.X, ALU.add)
        nc.vector.tensor_scalar_mul(mean3, mean3, 1.0 / N)
        nc.vector.tensor_scalar_mul(ex23, ex23, 1.0 / N)
        s3, t3 = _gn_coeffs(tc, nc, stats_pool, 128, 8, mean3,
                            ex23, vecs["b3"], vecs["g3"], vecs["t3"],
                            *ind128g8, name="gn3", eps_ap=eps_t)
        _elu1_affine_inplace(nc, pc, v0, s3, t3)

        for hh in range(2):
            o0 = pc.tile([128, N], F32, tag="o0")
            acco = stats_pool.tile([128, N // 512], F32, tag=f"acco{hh}")
            accosq = stats_pool.tile([128, N // 512], F32, tag=f"accosq{hh}")
            for cch in range(N // 512):
                sl = slice(cch * 512, (cch + 1) * 512)
                o_ps = psc.tile([128, 512], F32, tag="o_ps")
                nc.tensor.matmul(o_ps, w4T[:, hh * 128:(hh + 1) * 128],
                                 v0[:, sl], start=True, stop=True)
                nc.scalar.activation(o0[:, sl], o_ps, AF.Identity,
                                     accum_out=acco[:, cch:cch + 1])
                nc.scalar.activation(sqscr, o0[:, sl], AF.Square,
                                     accum_out=accosq[:, cch:cch + 1])
            mean4 = stats_pool.tile([128, 1], F32, tag=f"mean4_{hh}")
            ex24 = stats_pool.tile([128, 1], F32, tag=f"ex24_{hh}")
            nc.vector.tensor_reduce(mean4, acco, AX.X, ALU.add)
            nc.vector.tensor_reduce(ex24, accosq, AX.X, ALU.add)
            nc.vector.tensor_scalar_mul(mean4, mean4, 1.0 / N)
            nc.vector.tensor_scalar_mul(ex24, ex24, 1.0 / N)
            s4, t4 = _gn_coeffs(
                tc, nc, stats_pool, 128, 4, mean4, ex24,
                vecs["b4"][:, hh:hh + 1],
                vecs["g4"][:, hh:hh + 1],
                vecs["t4"][:, hh:hh + 1],
                *ind128g4, name=f"gn4_{hh}", eps_ap=eps_t)
            _elu1_affine_inplace(nc, pc, o0, s4, t4)
            nc.vector.tensor_scalar_add(o0, o0, -1.0)
            nc.sync.dma_start(out_d[hh * 128:(hh + 1) * 128, :], o0)

    persist.release()
    stats_pool.release()
    consts.release()


# ---------------- host-side prep ----------------

def _inds(Cc, Gg):
    gs = Cc // Gg
    indT = np.zeros((Cc, Gg), np.float32)
    bindT = np.zeros((Gg, Cc), np.float32)
    for g in range(Gg):
        indT[g * gs:(g + 1) * gs, g] = 1.0 / gs
        bindT[g, g * gs:(g + 1) * gs] = 1.0
    return indT, bindT


def _w1big(w1b, w1cT):
    big = np.zeros((68, 32 * 16), np.float32)
    for k in range(16):
        big[4 * k:4 * k + 3, 32 * k:32 * (k + 1)] = w1b.T
        big[64:68, 32 * k:32 * (k + 1)] = w1cT
    return big


def _split3(x):
    h = x.astype(ml_dtypes.bfloat16)
    r = x.astype(np.float32) - h.astype(np.float32)
    m = r.astype(ml_dtypes.bfloat16)
    l = (r - m.astype(np.float32)).astype(ml_dtypes.bfloat16)
    return h, m, l


def _qb_cb(points_b):
    p = points_b.astype(np.float64)
    sq = (p * p).sum(0)
    h, m, l = _split3(points_b.astype(np.float32))
    sh, sm, sl = _split3(sq.astype(np.float32))
    BFD = ml_dtypes.bfloat16
    ones = np.ones_like(h[0:1])
    qb = np.concatenate([h, h, m, m, h, l, ones, ones, ones], 0)
    cb = np.concatenate([(2 * h.astype(np.float32)).astype(BFD),
                         (2 * m.astype(np.float32)).astype(BFD),
                         (2 * h.astype(np.float32)).astype(BFD),
                         (2 * m.astype(np.float32)).astype(BFD),
                         (2 * l.astype(np.float32)).astype(BFD),
                         (2 * h.astype(np.float32)).astype(BFD),
                         -sh[None, :], -sm[None, :], -sl[None, :]], 0)
    return np.ascontiguousarray(qb), np.ascontiguousarray(cb)


def prep_A(points_b, weights, half, R):
    """Inputs for one phase-A core: query rows [half*R, (half+1)*R)."""
    N = points_b.shape[1]
    w1 = np.asarray(weights["w2d_0"])
    b1 = np.asarray(weights["b2d_0"])
    w1a, w1b = w1[:, 0:3], w1[:, 3:6]
    w1cT = np.concatenate([(w1a - w1b).T, b1[None, :]], 0).astype(np.float32)
    qb, cb = _qb_cb(points_b)
    sl = slice(half * R, (half + 1) * R)
    pts = points_b.astype(np.float32)
    ptsq = np.concatenate([pts[:, sl], np.ones((1, R), np.float32)], 0)
    tbl = np.concatenate([pts.T, np.zeros((N, 1), np.float32)], 1)
    return {
        "pts": np.ascontiguousarray(ptsq),
        "qb": np.ascontiguousarray(qb[:, sl]),
        "cb": cb,
        "tbl": np.ascontiguousarray(tbl),
        "w1bTg": np.ascontiguousarray(_w1big(w1b, w1cT)),
    }


def prep_B(x1s, s1, t1, weights):
    m = {
        "x1s": x1s,
        "s1": s1.reshape(-1, 1).astype(np.float32),
        "t1": t1.reshape(-1, 1).astype(np.float32),
        "w2T": np.ascontiguousarray(np.asarray(weights["w2d_1"]).T.astype(np.float32)),
        "w3T": np.ascontiguousarray(np.asarray(weights["w1d_0"]).T.astype(np.float32)),
        "w4T": np.ascontiguousarray(np.asarray(weights["w1d_1"]).T.astype(np.float32)),
        "b2": (np.asarray(weights["b2d_1"]) - np.asarray(weights["w2d_1"]).sum(1)).reshape(-1, 1).astype(np.float32),
        "g2": np.asarray(weights["g2d_1"]).reshape(-1, 1).astype(np.float32),
        "t2": np.asarray(weights["t2d_1"]).reshape(-1, 1).astype(np.float32),
        "b3": (np.asarray(weights["b1d_0"]) - np.asarray(weights["w1d_0"]).sum(1)).reshape(-1, 1).astype(np.float32),
        "g3": np.asarray(weights["g1d_0"]).reshape(-1, 1).astype(np.float32),
        "t3": np.asarray(weights["t1d_0"]).reshape(-1, 1).astype(np.float32),
        "b4": np.ascontiguousarray((np.asarray(weights["b1d_1"]) - np.asarray(weights["w1d_1"]).sum(1)).reshape(2, 128).T.astype(np.float32)),
        "g4": np.ascontiguousarray(np.asarray(weights["g1d_1"]).reshape(2, 128).T.astype(np.float32)),
        "t4": np.ascontiguousarray(np.asarray(weights["t1d_1"]).reshape(2, 128).T.astype(np.float32)),
    }
    for nm, (cc, gg) in (("c64", (64, 8)), ("c128", (128, 8)),
                         ("c128h", (128, 4))):
        indT, bindT = _inds(cc, gg)
        m[f"indT_{nm}"] = indT
        m[f"bindT_{nm}"] = bindT
    return m


def host_gn1(accx1, accx1sq, g1, t1g, NK):
    """Host-side GN1 scale/shift from merged per-channel partial sums."""
    mean1 = accx1.sum(1, dtype=np.float64) / NK
    ex2 = accx1sq.sum(1, dtype=np.float64) / NK
    gm = mean1.reshape(8, 4).mean(1)
    gex2 = ex2.reshape(8, 4).mean(1)
    var = gex2 - gm * gm
    s = np.asarray(g1, np.float64) / np.sqrt(var.repeat(4) + EPS)
    t = np.asarray(t1g, np.float64) - s * gm.repeat(4)
    return s.astype(np.float32), t.astype(np.float32)


_CACHE = {}


def _build_A(N, R, ch=256):
    key = ("A", N, R, ch)
    if key in _CACHE:
        return _CACHE[key]
    nc = bacc.Bacc("TRN2", target_bir_lowering=False, debug=False,
                   num_devices=8)
    NTh = R // 128
    ins = {
        "pts": nc.dram_tensor("pts", [4, R], F32, kind="ExternalInput").ap(),
        "qb": nc.dram_tensor("qb", [21, R], BF, kind="ExternalInput").ap(),
        "cb": nc.dram_tensor("cb", [21, N], BF, kind="ExternalInput").ap(),
        "tbl": nc.dram_tensor("tbl", [N, 4], F32, kind="ExternalInput").ap(),
        "w1bTg": nc.dram_tensor("w1bTg", [68, 32 * 16], F32,
                                kind="ExternalInput").ap(),
    }
    outs = {
        "x1o": nc.dram_tensor("x1o", [NTh, 32, 128 * K], F32,
                              kind="ExternalOutput").ap(),
        "accx1o": nc.dram_tensor("accx1o", [32, NTh * 4], F32,
                                 kind="ExternalOutput").ap(),
        "accx1sqo": nc.dram_tensor("accx1sqo", [32, NTh * 4], F32,
                                   kind="ExternalOutput").ap(),
    }
    with tile.TileContext(nc) as tc:
        build_phaseA(tc, outs, ins, N, R, ch)
    nc.compile()
    _CACHE[key] = nc
    return nc


def _build_BC(N, ch=256):
    key = ("BC", N, ch)
    if key in _CACHE:
        return _CACHE[key]
    nc = bacc.Bacc("TRN2", target_bir_lowering=False, debug=False,
                   num_devices=8)
    NT = N // 128
    ins = {"x1s": nc.dram_tensor("x1s", [NT, 32, 128 * K], F32,
                                 kind="ExternalInput").ap()}
    specs = [("w2T", [32, 64]), ("w3T", [64, 128]), ("w4T", [128, 256]),
             ("s1", [32, 1]), ("t1", [32, 1]),
             ("b2", [64, 1]), ("g2", [64, 1]), ("t2", [64, 1]),
             ("b3", [128, 1]), ("g3", [128, 1]), ("t3", [128, 1]),
             ("b4", [128, 2]), ("g4", [128, 2]), ("t4", [128, 2]),
             ("indT_c64", [64, 8]), ("bindT_c64", [8, 64]),
             ("indT_c128", [128, 8]), ("bindT_c128", [8, 128]),
             ("indT_c128h", [128, 4]), ("bindT_c128h", [4, 128])]
    for nm, shape in specs:
        ins[nm] = nc.dram_tensor(nm, shape, F32, kind="ExternalInput").ap()
    outs = {"out": nc.dram_tensor("out", [256, N], F32,
                                  kind="ExternalOutput").ap()}
    with tile.TileContext(nc) as tc:
        build_phaseBC(tc, outs, ins, N, ch)
    nc.compile()
    _CACHE[key] = nc
    return nc


def kernel(points, _trace=False, **weights):
    points = np.asarray(points)
    Bn, _, N = points.shape
    R = N // 2
    ch = 256 if N >= 2048 else 64
    ncA = _build_A(N, R, ch)
    ncB = _build_BC(N, ch)

    tkw = {}
    if _trace:
        import tempfile
        tkw = dict(trace=True)

    in_maps_A = []
    for c in range(8):
        in_maps_A.append(prep_A(points[(c // 2) % Bn], weights, c % 2, R))
    resA = run_bass_kernel_spmd(ncA, in_maps_A, core_ids=list(range(8)),
                                **({**tkw, "tmpdir": tempfile.mkdtemp(prefix="trA_")} if _trace else {}))

    g1 = np.asarray(weights["g2d_0"])
    t1g = np.asarray(weights["t2d_0"])
    in_maps_B = []
    per_batch = []
    for b in range(Bn):
        e, o = resA.results[2 * b], resA.results[2 * b + 1]
        x1s = np.ascontiguousarray(
            np.concatenate([e["x1o"], o["x1o"]], 0))
        acc = np.concatenate([e["accx1o"], o["accx1o"]], 1)
        accsq = np.concatenate([e["accx1sqo"], o["accx1sqo"]], 1)
        s1, t1 = host_gn1(acc, accsq, g1, t1g, N * K)
        per_batch.append(prep_B(x1s, s1, t1, weights))
    for c in range(8):
        in_maps_B.append(per_batch[c % Bn])
    resB = run_bass_kernel_spmd(ncB, in_maps_B, core_ids=list(range(8)),
                                **({**tkw, "tmpdir": tempfile.mkdtemp(prefix="trB_")} if _trace else {}))
    out = np.stack([resB.results[b]["out"] for b in range(Bn)], 0)
    if _trace:
        kernel.last_exec_A = resA.exec_time_ns
        kernel.last_exec_B = resB.exec_time_ns
        kernel.last_exec_ns = (resA.exec_time_ns or 0) + (resB.exec_time_ns or 0)
        kernel.trace_A = resA.instructions_and_trace
        kernel.trace_B = resB.instructions_and_trace
    return out.astype(np.float32)



# revision 6
# speedup vs baseline: 1.5412x; 1.5412x over previous
"""Trainium2 Bass kernel for nn_AbsoluteRelativePositionEmbedding.

Three SPMD launches over 8 NeuronCores:
  Launch A (8 cores): each core handles HALF of one batch's query rows.
    kNN(16) over all N=8192 candidates -> gather neighbor coords ->
    conv1(6->32) -> x1 + per-channel partial sums (for GroupNorm-1 stats).
  Host: merges the tiny GN1 partial sums per batch -> scale/shift vectors.
  Launch B (8 cores, row-split): each core processes its OWN phase-A rows:
    gn1+ELU -> conv2(32->64, block-diag float32r) -> max over K -> ypool
    half + GN2 partial sums. Elementwise work is packed 4 row-tiles deep so
    all 128 partitions are busy.
  Host: merges GN2 partials -> s2/t2.
  Launch C (8 cores): full batch per core, channel-split final conv:
    gn2+ELU -> conv3(64->128) -> gn3 (on device) + ELU -> conv4 half
    (128 of 256 out channels) -> gn4 (on device) + ELU -> out half.

The distance matmul uses a 3-level bf16 split (21 contraction rows) which
matches fp32 accuracy at 4x the PE rate. ELU(x)+1 = min(exp(x),1)+relu(x)
is used internally; the +1 is folded into the next conv's bias.
"""
import sys
sys.path.insert(0, '/opt/trn_rl_repo')

import numpy as np
import ml_dtypes

import concourse.bass as bass
import concourse.bacc as bacc
import concourse.mybir as mybir
import concourse.tile as tile
from concourse.bass import IndirectOffsetOnAxis
from concourse.bass_utils import run_bass_kernel_spmd
from concourse.masks import make_identity

F32 = mybir.dt.float32
F32R = mybir.dt.float32r
U32 = mybir.dt.uint32
BF = mybir.dt.bfloat16
AF = mybir.ActivationFunctionType
ALU = mybir.AluOpType
AX = mybir.AxisListType

K = 16
EPS = 1e-5
NEG = -1e30
GROUPS = 8


def _r(ap):
    """float32r view for full-rate fp32 matmuls."""
    return ap.bitcast(F32R)


def _elu1_affine_inplace(nc, pool, x, s, t):
    """x = elu(s*x + t) + 1 in place, using shared [128, f] scratch tags."""
    p, f = x.shape[0], x.shape[1]
    m = pool.tile([128, f], F32, tag="elu_scr_m")
    e = pool.tile([128, f], F32, tag="elu_scr_e")
    nc.scalar.activation(m[0:p, :], x, AF.Relu, scale=s, bias=t)
    nc.scalar.activation(e[0:p, :], x, AF.Exp, scale=s, bias=t)
    nc.vector.scalar_tensor_tensor(x, e[0:p, :], 1.0, m[0:p, :], ALU.min,
                                   ALU.add)


def _gn_coeffs(tc, nc, stats_pool, C, G, mean_c, ex2_c, bvec, gvec, tvec,
               indT, bindT, name, eps_ap):
    """Per-channel GN scale s / shift t so that gn(y+b) = s*y + t."""
    psp = tc.alloc_tile_pool(name=f"ps_{name}", bufs=1, space="PSUM")
    m1 = stats_pool.tile([C, 1], F32, tag=f"m1_{name}")
    m2 = stats_pool.tile([C, 1], F32, tag=f"m2_{name}")
    nc.vector.tensor_add(m1, mean_c, bvec)
    tmp = stats_pool.tile([C, 1], F32, tag=f"tmp_{name}")
    nc.vector.tensor_add(tmp, mean_c, m1)
    nc.vector.tensor_mul(tmp, tmp, bvec)
    nc.vector.tensor_add(m2, ex2_c, tmp)
    gm_ps = psp.tile([G, 1], F32, tag="gm_ps")
    gm_sb = stats_pool.tile([G, 1], F32, tag=f"gm_sb_{name}")
    bc1 = stats_pool.tile([C, 1], F32, tag=f"bc1_{name}")
    bc2 = stats_pool.tile([C, 1], F32, tag=f"bc2_{name}")
    bc_ps = psp.tile([C, 1], F32, tag="bc_ps")
    nc.tensor.matmul(gm_ps, indT, m1, start=True, stop=True)
    nc.scalar.copy(gm_sb, gm_ps)
    nc.tensor.matmul(bc_ps, bindT, gm_sb, start=True, stop=True)
    nc.scalar.copy(bc1, bc_ps)
    nc.tensor.matmul(gm_ps, indT, m2, start=True, stop=True)
    nc.scalar.copy(gm_sb, gm_ps)
    nc.tensor.matmul(bc_ps, bindT, gm_sb, start=True, stop=True)
    nc.scalar.copy(bc2, bc_ps)
    var = stats_pool.tile([C, 1], F32, tag=f"var_{name}")
    nc.vector.tensor_mul(tmp, bc1, bc1)
    nc.vector.tensor_sub(var, bc2, tmp)
    sd = stats_pool.tile([C, 1], F32, tag=f"sd_{name}")
    nc.scalar.activation(sd, var, AF.Sqrt, bias=eps_ap[0:C, :])
    s = stats_pool.tile([C, 1], F32, tag=f"s_{name}")
    nc.vector.reciprocal(s, sd)
    nc.vector.tensor_mul(s, s, gvec)
    t = stats_pool.tile([C, 1], F32, tag=f"t_{name}")
    nc.vector.tensor_sub(tmp, bc1, bvec)
    nc.vector.tensor_mul(tmp, s, tmp)
    nc.vector.tensor_sub(t, tvec, tmp)
    psp.release()
    return s, t


def build_phaseA(tc, outs, ins, N, R, ch=512):
    """kNN + gather + conv1 for R query rows against N candidates."""
    nc = tc.nc
    NTh = R // 128
    NCH = N // ch

    tbl_d = ins["tbl"]
    x1o_d = outs["x1o"]

    consts = tc.alloc_tile_pool(name="consts", bufs=1)
    persist = tc.alloc_tile_pool(name="persist", bufs=1)

    w1bTg = consts.tile([68, 32 * 16], F32, tag="w1bTg")
    nc.sync.dma_start(w1bTg, ins["w1bTg"])
    ident = consts.tile([128, 128], F32, tag="ident")
    make_identity(nc, ident)

    Q = persist.tile([4, R], F32, tag="Q")
    Qb = persist.tile([21, R], BF, tag="Qb")
    Cb = persist.tile([21, N], BF, tag="Cb")
    nc.sync.dma_start(Q, ins["pts"])
    nc.sync.dma_start(Qb, ins["qb"])
    nc.sync.dma_start(Cb, ins["cb"])

    accx1 = persist.tile([32, NTh * 4], F32, tag="accx1")
    accx1sq = persist.tile([32, NTh * 4], F32, tag="accx1sq")

    with tc.tile_pool(name="pa_sb", bufs=2) as pa, \
         tc.tile_pool(name="pa_nd", bufs=2) as pa_nd, \
         tc.tile_pool(name="pa_ps", bufs=2, space="PSUM") as psa, \
         tc.tile_pool(name="pa_ps1", bufs=2, space="PSUM") as psa1, \
         tc.tile_pool(name="pa_pst", bufs=2, space="PSUM") as psat:
        for t in range(NTh):
            r0 = t * 128
            ndsb = pa_nd.tile([128, N], F32, tag="ndsb")
            CW = min(1024, N)
            for cc in range(N // CW):
                nd_ps = psa.tile([128, CW], F32, tag="nd_ps")
                for h in range(CW // 512):
                    nc.tensor.matmul(
                        nd_ps[:, h * 512:(h + 1) * 512],
                        Qb[:, r0:r0 + 128],
                        Cb[:, cc * CW + h * 512:cc * CW + (h + 1) * 512],
                        start=True, stop=True)
                nc.scalar.copy(ndsb[:, cc * CW:(cc + 1) * CW], nd_ps)
            # top-8 per chunk
            cand = pa.tile([128, NCH * 8], F32, tag="cand")
            for cq in range(NCH):
                nc.vector.max(out=cand[:, cq * 8:(cq + 1) * 8],
                              in_=ndsb[:, cq * ch:(cq + 1) * ch])
            r1 = pa.tile([128, 8], F32, tag="r1")
            r2 = pa.tile([128, 8], F32, tag="r2")
            r3 = pa.tile([128, 8], F32, tag="r3")
            cand2 = pa.tile([128, NCH * 8], F32, tag="cand2")
            cand3 = pa.tile([128, NCH * 8], F32, tag="cand3")
            nc.vector.max(out=r1, in_=cand)
            nc.vector.match_replace(out=cand2, in_to_replace=r1,
                                    in_values=cand, imm_value=NEG)
            nc.vector.max(out=r2, in_=cand2)
            nc.vector.match_replace(out=cand3, in_to_replace=r2,
                                    in_values=cand2, imm_value=NEG)
            nc.vector.max(out=r3, in_=cand3)
            # winners: ranks 2..17 (rank 1 is self)
            wA = pa.tile([128, 8], F32, tag="wA")
            nc.vector.tensor_copy(wA[:, 0:7], r1[:, 1:8])
            nc.vector.tensor_copy(wA[:, 7:8], r3[:, 0:1])
            gidx = pa.tile([128, K], U32, tag="gidx")
            nc.vector.max_index(gidx[:, 0:8], wA, ndsb)
            nc.vector.max_index(gidx[:, 8:16], r2, ndsb)
            # gather neighbor coords (16B rows from the xyz0 table)
            gt = pa.tile([128, K * 4], F32, tag="gt")
            for k in range(K):
                nc.gpsimd.indirect_dma_start(
                    out=gt[:, 4 * k:4 * (k + 1)], out_offset=None,
                    in_=tbl_d,
                    in_offset=IndirectOffsetOnAxis(ap=gidx[:, k:k + 1],
                                                   axis=0))
            gtT_ps = psat.tile([64, 128], F32, tag="gtT_ps")
            nc.tensor.transpose(gtT_ps, gt, ident)
            rhs68 = pa.tile([68, 128], F32, tag="rhs68")
            nc.scalar.copy(rhs68[0:64, :], gtT_ps)
            nc.scalar.copy(rhs68[64:68, :], Q[:, r0:r0 + 128])
            # conv1 into (32, 128*K), position order (k, r)
            x1sb = pa.tile([32, 128 * K], F32, tag="x1sb")
            for q in range(K // 4):
                x1_ps = psa1.tile([32, 512], F32, tag="x1_ps")
                for j in range(4):
                    k = 4 * q + j
                    nc.tensor.matmul(x1_ps[:, j * 128:(j + 1) * 128],
                                     w1bTg[:, 32 * k:32 * (k + 1)], rhs68,
                                     start=True, stop=True)
                sl = slice(q * 512, (q + 1) * 512)
                nc.scalar.activation(
                    x1sb[:, sl], x1_ps, AF.Identity,
                    accum_out=accx1[:, 4 * t + q:4 * t + q + 1])
                sqsc = pa.tile([32, 512], F32, tag="sqsc")
                nc.scalar.activation(
                    sqsc, x1sb[:, sl], AF.Square,
                    accum_out=accx1sq[:, 4 * t + q:4 * t + q + 1])
            nc.sync.dma_start(x1o_d[t], x1sb)

    nc.sync.dma_start(outs["accx1o"], accx1)
    nc.sync.dma_start(outs["accx1sqo"], accx1sq)
    persist.release()
    consts.release()


def build_phaseB(tc, outs, ins, R):
    """Row-split: gn1+ELU -> conv2 (block-diag) -> max over K -> ypool half.

    x1 tiles are packed 4-deep on partitions: partition 32j+c holds channel
    c of row-tile 4t+j. conv2 runs as two block-diagonal (64->128) matmuls.
    Emits per-core GN2 partial sums (z sums, y^2 sums) for the host merge.
    """
    nc = tc.nc
    NP = R // 512                  # packed tiles (4 row-tiles each)

    x1s_d = ins["x1s"]
    yp_d = outs["ypoolo"]

    consts = tc.alloc_tile_pool(name="consts", bufs=1)
    persist = tc.alloc_tile_pool(name="persist", bufs=1)

    s1t = consts.tile([128, 1], F32, tag="s1t")
    t1t = consts.tile([128, 1], F32, tag="t1t")
    w2bdf = consts.tile([128, 128], F32, tag="w2bdf")
    w2bd = consts.tile([128, 128], F32R, tag="w2bd")
    nc.sync.dma_start(s1t, ins["s1t"])
    nc.sync.dma_start(t1t, ins["t1t"])
    nc.sync.dma_start(w2bdf, ins["w2bd"])
    nc.scalar.copy(w2bd, w2bdf)

    accz = persist.tile([128, NP], F32, tag="accz")
    accysq = persist.tile([128, 2 * NP], F32, tag="accysq")

    with tc.tile_pool(name="pb_sb", bufs=2) as pb, \
         tc.tile_pool(name="pb_scr", bufs=1) as pbs, \
         tc.tile_pool(name="pb_ps", bufs=2, space="PSUM") as psb:
        for t in range(NP):
            xp = pb.tile([128, 2048], F32, tag="xp")
            for j in range(4):
                nc.sync.dma_start(xp[32 * j:32 * (j + 1), :],
                                  x1s_d[4 * t + j])
            m_ = pbs.tile([128, 2048], F32, tag="elu_m")
            e_ = pbs.tile([128, 2048], F32, tag="elu_e")
            nc.scalar.activation(m_, xp, AF.Relu, scale=s1t, bias=t1t)
            nc.scalar.activation(e_, xp, AF.Exp, scale=s1t, bias=t1t)
            z = pb.tile([128, 2048], F32R, tag="z")
            # z = elu(gn1(x1)) + 1  (the +1 is folded into b2' on host)
            nc.vector.scalar_tensor_tensor(z, e_, 1.0, m_, ALU.min, ALU.add,
                                           accum_out=accz[:, t:t + 1])
            sqscr = pbs.tile([128, 2048], F32, tag="sqscr")
            for h in range(2):     # row-tiles (4t+2h, 4t+2h+1)
                y_ps = psb.tile([128, 2048], F32, tag="y_ps")
                for q in range(4):
                    sl = slice(q * 512, (q + 1) * 512)
                    nc.tensor.matmul(y_ps[:, sl],
                                     w2bd[64 * h:64 * (h + 1), :],
                                     z[64 * h:64 * (h + 1), sl],
                                     start=True, stop=True)
                nc.scalar.activation(sqscr, y_ps, AF.Square,
                                     accum_out=accysq[:, 2 * t + h:2 * t + h + 1])
                # max over K (position order (k, r): fold k halves)
                p1 = pbs.tile([128, 1024], F32, tag="p1")
                p2 = pbs.tile([128, 512], F32, tag="p2")
                p3 = pbs.tile([128, 256], F32, tag="p3")
                p4 = pb.tile([128, 128], F32, tag="p4")
                yh = pbs.tile([128, 1024], F32, tag="yh")
                nc.scalar.copy(yh, y_ps[:, 1024:2048])
                nc.vector.tensor_tensor(p1, y_ps[:, 0:1024],
                                        yh, op=ALU.max)
                nc.vector.tensor_tensor(p2, p1[:, 0:512], p1[:, 512:1024],
                                        op=ALU.max)
                nc.vector.tensor_tensor(p3, p2[:, 0:256], p2[:, 256:512],
                                        op=ALU.max)
                nc.vector.tensor_tensor(p4, p3[:, 0:128], p3[:, 128:256],
                                        op=ALU.max)
                rt = 4 * t + 2 * h
                nc.sync.dma_start(yp_d[:, rt * 128:(rt + 1) * 128],
                                  p4[0:64, :])
                nc.sync.dma_start(yp_d[:, (rt + 1) * 128:(rt + 2) * 128],
                                  p4[64:128, :])

    nc.sync.dma_start(outs["acczo"], accz)
    nc.sync.dma_start(outs["accysqo"], accysq)
    persist.release()
    consts.release()


def build_phaseC(tc, outs, ins, N):
    """Full batch: gn2+ELU -> conv3 -> gn3+ELU -> conv4 half -> gn4+ELU.

    ypool is packed 2-deep on partitions (partition 64h+c = channel c for
    column half h). conv4 computes only this core's 128 of 256 channels;
    its 4 GN groups are self-contained so gn4 runs fully on device.
    """
    nc = tc.nc
    NH = N // 2

    out_d = outs["out"]

    consts = tc.alloc_tile_pool(name="consts", bufs=1)
    stats_pool = tc.alloc_tile_pool(name="stats", bufs=1)
    persist = tc.alloc_tile_pool(name="persist", bufs=1)

    def load_const(name, shape):
        t = consts.tile(shape, F32, tag=name)
        nc.sync.dma_start(t, ins[name])
        return t

    s2t = load_const("s2t", [128, 1])
    t2t = load_const("t2t", [128, 1])
    w3T2f = load_const("w3T2", [128, 128])
    w4hf = load_const("w4h", [128, 128])
    w3T2 = consts.tile([128, 128], F32R, tag="w3T2r")
    w4h = consts.tile([128, 128], F32R, tag="w4hr")
    nc.scalar.copy(w3T2, w3T2f)
    nc.scalar.copy(w4h, w4hf)
    b3 = load_const("b3", [128, 1])
    g3 = load_const("g3", [128, 1])
    t3 = load_const("t3", [128, 1])
    b4 = load_const("b4", [128, 1])
    g4 = load_const("g4", [128, 1])
    t4 = load_const("t4", [128, 1])
    ind3T = load_const("indT_c128", [128, 8])
    bind3T = load_const("bindT_c128", [8, 128])
    ind4T = load_const("indT_c128h", [128, 4])
    bind4T = load_const("bindT_c128h", [4, 128])
    eps_t = consts.tile([128, 1], F32, tag="eps_t")
    nc.vector.memset(eps_t, EPS)

    with tc.tile_pool(name="pc_sb", bufs=1) as pc, \
         tc.tile_pool(name="pc_ps", bufs=2, space="PSUM") as psc:
        yp = pc.tile([128, NH], F32, tag="yp")
        nc.sync.dma_start(yp[0:64, :], ins["ypool"][:, 0:NH])
        nc.sync.dma_start(yp[64:128, :], ins["ypool"][:, NH:N])
        u = pc.tile([128, NH], F32R, tag="u")
        m0 = pc.tile([128, NH], F32, tag="elu_scr_m")
        e0 = pc.tile([128, NH], F32, tag="elu_scr_e")
        nc.scalar.activation(m0, yp, AF.Relu, scale=s2t, bias=t2t)
        nc.scalar.activation(e0, yp, AF.Exp, scale=s2t, bias=t2t)
        nc.vector.scalar_tensor_tensor(u, e0, 1.0, m0, ALU.min, ALU.add)

        NCH = NH // 512
        vs = []
        accv = stats_pool.tile([128, 2 * NCH], F32, tag="accv")
        accvsq = stats_pool.tile([128, 2 * NCH], F32, tag="accvsq")
        sqscr = pc.tile([128, 512], F32, tag="sqscr")
        for hh in range(2):
            v = pc.tile([128, NH], F32R, tag=f"v{hh}")
            for cch in range(NCH):
                sl = slice(cch * 512, (cch + 1) * 512)
                v_ps = psc.tile([128, 512], F32, tag="v_ps")
                nc.tensor.matmul(v_ps, w3T2[64 * hh:64 * (hh + 1), :],
                                 u[64 * hh:64 * (hh + 1), sl],
                                 start=True, stop=True)
                ci = hh * NCH + cch
                nc.scalar.activation(v[:, sl], v_ps, AF.Identity,
                                     accum_out=accv[:, ci:ci + 1])
                nc.scalar.activation(sqscr, v.bitcast(F32)[:, sl], AF.Square,
                                     accum_out=accvsq[:, ci:ci + 1])
            vs.append(v)
        mean3 = stats_pool.tile([128, 1], F32, tag="mean3")
        ex23 = stats_pool.tile([128, 1], F32, tag="ex23")
        nc.vector.tensor_reduce(mean3, accv, AX.X, ALU.add)
        nc.vector.tensor_reduce(ex23, accvsq, AX.X, ALU.add)
        nc.vector.tensor_scalar_mul(mean3, mean3, 1.0 / N)
        nc.vector.tensor_scalar_mul(ex23, ex23, 1.0 / N)
        s3, t3c = _gn_coeffs(tc, nc, stats_pool, 128, GROUPS, mean3, ex23,
                             b3, g3, t3, ind3T, bind3T, name="gn3",
                             eps_ap=eps_t)
        for v in vs:
            mv = pc.tile([128, NH], F32, tag="elu_scr_m")
            ev = pc.tile([128, NH], F32, tag="elu_scr_e")
            nc.scalar.activation(mv, v.bitcast(F32), AF.Relu, scale=s3,
                                 bias=t3c)
            nc.scalar.activation(ev, v.bitcast(F32), AF.Exp, scale=s3,
                                 bias=t3c)
            nc.vector.scalar_tensor_tensor(v, ev, 1.0, mv, ALU.min, ALU.add)

        os_ = []
        acco = stats_pool.tile([128, 2 * NCH], F32, tag="acco")
        accosq = stats_pool.tile([128, 2 * NCH], F32, tag="accosq")
        for hh in range(2):
            o = pc.tile([128, NH], F32, tag=f"o{hh}")
            for cch in range(NCH):
                sl = slice(cch * 512, (cch + 1) * 512)
                o_ps = psc.tile([128, 512], F32, tag="o_ps")
                nc.tensor.matmul(o_ps, w4h, vs[hh][:, sl],
                                 start=True, stop=True)
                ci = hh * NCH + cch
                nc.scalar.activation(o[:, sl], o_ps, AF.Identity,
                                     accum_out=acco[:, ci:ci + 1])
                nc.scalar.activation(sqscr, o[:, sl], AF.Square,
                                     accum_out=accosq[:, ci:ci + 1])
            os_.append(o)
        mean4 = stats_pool.tile([128, 1], F32, tag="mean4")
        ex24 = stats_pool.tile([128, 1], F32, tag="ex24")
        nc.vector.tensor_reduce(mean4, acco, AX.X, ALU.add)
        nc.vector.tensor_reduce(ex24, accosq, AX.X, ALU.add)
        nc.vector.tensor_scalar_mul(mean4, mean4, 1.0 / N)
        nc.vector.tensor_scalar_mul(ex24, ex24, 1.0 / N)
        s4, t4c = _gn_coeffs(tc, nc, stats_pool, 128, 4, mean4, ex24,
                             b4, g4, t4, ind4T, bind4T, name="gn4",
                             eps_ap=eps_t)
        for hh in range(2):
            o = os_[hh]
            _elu1_affine_inplace(nc, pc, o, s4, t4c)
            nc.vector.tensor_scalar_add(o, o, -1.0)
            nc.sync.dma_start(out_d[:, hh * NH:(hh + 1) * NH], o)

    persist.release()
    stats_pool.release()
    consts.release()


# ---------------- host-side prep ----------------

def _inds(Cc, Gg):
    gs = Cc // Gg
    indT = np.zeros((Cc, Gg), np.float32)
    bindT = np.zeros((Gg, Cc), np.float32)
    for g in range(Gg):
        indT[g * gs:(g + 1) * gs, g] = 1.0 / gs
        bindT[g, g * gs:(g + 1) * gs] = 1.0
    return indT, bindT


def _w1big(w1b, w1cT):
    big = np.zeros((68, 32 * 16), np.float32)
    for k in range(16):
        big[4 * k:4 * k + 3, 32 * k:32 * (k + 1)] = w1b.T
        big[64:68, 32 * k:32 * (k + 1)] = w1cT
    return big


def _split3(x):
    h = x.astype(ml_dtypes.bfloat16)
    r = x.astype(np.float32) - h.astype(np.float32)
    m = r.astype(ml_dtypes.bfloat16)
    l = (r - m.astype(np.float32)).astype(ml_dtypes.bfloat16)
    return h, m, l


def _qb_cb(points_b):
    p = points_b.astype(np.float64)
    sq = (p * p).sum(0)
    h, m, l = _split3(points_b.astype(np.float32))
    sh, sm, sl = _split3(sq.astype(np.float32))
    BFD = ml_dtypes.bfloat16
    ones = np.ones_like(h[0:1])
    qb = np.concatenate([h, h, m, m, h, l, ones, ones, ones], 0)
    cb = np.concatenate([(2 * h.astype(np.float32)).astype(BFD),
                         (2 * m.astype(np.float32)).astype(BFD),
                         (2 * h.astype(np.float32)).astype(BFD),
                         (2 * m.astype(np.float32)).astype(BFD),
                         (2 * l.astype(np.float32)).astype(BFD),
                         (2 * h.astype(np.float32)).astype(BFD),
                         -sh[None, :], -sm[None, :], -sl[None, :]], 0)
    return np.ascontiguousarray(qb), np.ascontiguousarray(cb)


def prep_A(points_b, weights, half, R):
    """Inputs for one phase-A core: query rows [half*R, (half+1)*R)."""
    N = points_b.shape[1]
    w1 = np.asarray(weights["w2d_0"])
    b1 = np.asarray(weights["b2d_0"])
    w1a, w1b = w1[:, 0:3], w1[:, 3:6]
    w1cT = np.concatenate([(w1a - w1b).T, b1[None, :]], 0).astype(np.float32)
    qb, cb = _qb_cb(points_b)
    sl = slice(half * R, (half + 1) * R)
    pts = points_b.astype(np.float32)
    ptsq = np.concatenate([pts[:, sl], np.ones((1, R), np.float32)], 0)
    tbl = np.concatenate([pts.T, np.zeros((N, 1), np.float32)], 1)
    return {
        "pts": np.ascontiguousarray(ptsq),
        "qb": np.ascontiguousarray(qb[:, sl]),
        "cb": cb,
        "tbl": np.ascontiguousarray(tbl),
        "w1bTg": np.ascontiguousarray(_w1big(w1b, w1cT)),
    }


def prep_B(x1s, s1, t1, weights):
    w2T = np.asarray(weights["w2d_1"]).T.astype(np.float32)   # (32, 64)
    w2bd = np.zeros((128, 128), np.float32)
    for rep in range(2):
        w2bd[64 * rep + 0:64 * rep + 32, 0:64] = w2T
        w2bd[64 * rep + 32:64 * rep + 64, 64:128] = w2T
    return {
        "x1s": x1s,
        "s1t": np.ascontiguousarray(np.tile(s1.reshape(-1), 4).reshape(-1, 1).astype(np.float32)),
        "t1t": np.ascontiguousarray(np.tile(t1.reshape(-1), 4).reshape(-1, 1).astype(np.float32)),
        "w2bd": np.ascontiguousarray(w2bd),
    }


def host_gn1(accx1, accx1sq, g1, t1g, NK):
    """Host-side GN1 scale/shift from merged per-channel partial sums."""
    mean1 = accx1.sum(1, dtype=np.float64) / NK
    ex2 = accx1sq.sum(1, dtype=np.float64) / NK
    gm = mean1.reshape(8, 4).mean(1)
    gex2 = ex2.reshape(8, 4).mean(1)
    var = gex2 - gm * gm
    s = np.asarray(g1, np.float64) / np.sqrt(var.repeat(4) + EPS)
    t = np.asarray(t1g, np.float64) - s * gm.repeat(4)
    return s.astype(np.float32), t.astype(np.float32)


def host_gn2(acczs, accysqs, weights, NK):
    """GN2 scale/shift from the two cores' partial sums of one batch."""
    w2 = np.asarray(weights["w2d_1"], np.float64)            # (64, 32)
    b2f = (np.asarray(weights["b2d_1"], np.float64)
           - w2.sum(1))                                       # +1 fold
    zsum = np.zeros(32, np.float64)
    eysq = np.zeros(64, np.float64)
    for accz, accysq in zip(acczs, accysqs):
        zsum += accz.astype(np.float64).reshape(4, 32, -1).sum((0, 2))
        eysq += accysq.astype(np.float64).reshape(2, 64, -1).sum((0, 2))
    ymean = (w2 @ zsum) / NK
    mean_t = ymean + b2f
    e2_t = eysq / NK + 2.0 * b2f * ymean + b2f * b2f
    gm = mean_t.reshape(8, 8).mean(1)
    ge2 = e2_t.reshape(8, 8).mean(1)
    var = ge2 - gm * gm
    g2 = np.asarray(weights["g2d_1"], np.float64)
    t2 = np.asarray(weights["t2d_1"], np.float64)
    s = g2 / np.sqrt(var.repeat(8) + EPS)
    # out = g*((y+b2f) - gm)/sd + t2 = s*y + (t2 - s*gm + s*b2f)
    t = t2 - s * gm.repeat(8) + s * b2f
    return s.astype(np.float32), t.astype(np.float32)


def prep_C(ypool, s2, t2, weights, hh):
    w3T = np.asarray(weights["w1d_0"]).T.astype(np.float32)   # (64, 128)
    w3T2 = np.concatenate([w3T, w3T], 0)                      # (128, 128)
    w4T = np.asarray(weights["w1d_1"]).T.astype(np.float32)   # (128, 256)
    b3f = (np.asarray(weights["b1d_0"])
           - np.asarray(weights["w1d_0"]).sum(1))
    b4f = (np.asarray(weights["b1d_1"])
           - np.asarray(weights["w1d_1"]).sum(1))
    sl4 = slice(hh * 128, (hh + 1) * 128)
    m = {
        "ypool": np.ascontiguousarray(ypool),
        "s2t": np.ascontiguousarray(np.tile(s2, 2).reshape(-1, 1).astype(np.float32)),
        "t2t": np.ascontiguousarray(np.tile(t2, 2).reshape(-1, 1).astype(np.float32)),
        "w3T2": np.ascontiguousarray(w3T2),
        "w4h": np.ascontiguousarray(w4T[:, sl4]),
        "b3": b3f.reshape(-1, 1).astype(np.float32),
        "g3": np.asarray(weights["g1d_0"]).reshape(-1, 1).astype(np.float32),
        "t3": np.asarray(weights["t1d_0"]).reshape(-1, 1).astype(np.float32),
        "b4": np.ascontiguousarray(b4f[sl4].reshape(-1, 1).astype(np.float32)),
        "g4": np.ascontiguousarray(np.asarray(weights["g1d_1"])[sl4].reshape(-1, 1).astype(np.float32)),
        "t4": np.ascontiguousarray(np.asarray(weights["t1d_1"])[sl4].reshape(-1, 1).astype(np.float32)),
    }
    for nm, (cc, gg) in (("c128", (128, 8)), ("c128h", (128, 4))):
        indT, bindT = _inds(cc, gg)
        m[f"indT_{nm}"] = indT
        m[f"bindT_{nm}"] = bindT
    return m


_CACHE = {}


def _build_A(N, R, ch=512):
    key = ("A", N, R, ch)
    if key in _CACHE:
        return _CACHE[key]
    nc = bacc.Bacc("TRN2", target_bir_lowering=False, debug=False,
                   num_devices=8)
    NTh = R // 128
    ins = {
        "pts": nc.dram_tensor("pts", [4, R], F32, kind="ExternalInput").ap(),
        "qb": nc.dram_tensor("qb", [21, R], BF, kind="ExternalInput").ap(),
        "cb": nc.dram_tensor("cb", [21, N], BF, kind="ExternalInput").ap(),
        "tbl": nc.dram_tensor("tbl", [N, 4], F32, kind="ExternalInput").ap(),
        "w1bTg": nc.dram_tensor("w1bTg", [68, 32 * 16], F32,
                                kind="ExternalInput").ap(),
    }
    outs = {
        "x1o": nc.dram_tensor("x1o", [NTh, 32, 128 * K], F32,
                              kind="ExternalOutput").ap(),
        "accx1o": nc.dram_tensor("accx1o", [32, NTh * 4], F32,
                                 kind="ExternalOutput").ap(),
        "accx1sqo": nc.dram_tensor("accx1sqo", [32, NTh * 4], F32,
                                   kind="ExternalOutput").ap(),
    }
    with tile.TileContext(nc) as tc:
        build_phaseA(tc, outs, ins, N, R, ch)
    nc.compile()
    _CACHE[key] = nc
    return nc


def _build_B(R):
    key = ("B", R)
    if key in _CACHE:
        return _CACHE[key]
    nc = bacc.Bacc("TRN2", target_bir_lowering=False, debug=False,
                   num_devices=8)
    NTh = R // 128
    NP = R // 512
    ins = {
        "x1s": nc.dram_tensor("x1s", [NTh, 32, 128 * K], F32,
                              kind="ExternalInput").ap(),
        "s1t": nc.dram_tensor("s1t", [128, 1], F32, kind="ExternalInput").ap(),
        "t1t": nc.dram_tensor("t1t", [128, 1], F32, kind="ExternalInput").ap(),
        "w2bd": nc.dram_tensor("w2bd", [128, 128], F32,
                               kind="ExternalInput").ap(),
    }
    outs = {
        "ypoolo": nc.dram_tensor("ypoolo", [64, R], F32,
                                 kind="ExternalOutput").ap(),
        "acczo": nc.dram_tensor("acczo", [128, NP], F32,
                                kind="ExternalOutput").ap(),
        "accysqo": nc.dram_tensor("accysqo", [128, 2 * NP], F32,
                                  kind="ExternalOutput").ap(),
    }
    with tile.TileContext(nc) as tc:
        build_phaseB(tc, outs, ins, R)
    nc.compile()
    _CACHE[key] = nc
    return nc


def _build_C(N):
    key = ("C", N)
    if key in _CACHE:
        return _CACHE[key]
    nc = bacc.Bacc("TRN2", target_bir_lowering=False, debug=False,
                   num_devices=8)
    ins = {"ypool": nc.dram_tensor("ypool", [64, N], F32,
                                   kind="ExternalInput").ap()}
    specs = [("s2t", [128, 1]), ("t2t", [128, 1]), ("w3T2", [128, 128]),
             ("w4h", [128, 128]), ("b3", [128, 1]), ("g3", [128, 1]),
             ("t3", [128, 1]), ("b4", [128, 1]), ("g4", [128, 1]),
             ("t4", [128, 1]),
             ("indT_c128", [128, 8]), ("bindT_c128", [8, 128]),
             ("indT_c128h", [128, 4]), ("bindT_c128h", [4, 128])]
    for nm, shape in specs:
        ins[nm] = nc.dram_tensor(nm, shape, F32, kind="ExternalInput").ap()
    outs = {"out": nc.dram_tensor("out", [128, N], F32,
                                  kind="ExternalOutput").ap()}
    with tile.TileContext(nc) as tc:
        build_phaseC(tc, outs, ins, N)
    nc.compile()
    _CACHE[key] = nc
    return nc


def kernel(points, _trace=False, **weights):
    points = np.asarray(points)
    Bn, _, N = points.shape
    R = N // 2
    ncA = _build_A(N, R)
    ncB = _build_B(R)
    ncC = _build_C(N)

    if _trace:
        import tempfile

    def _run(ncX, in_maps, prefix):
        kw = {}
        if _trace:
            kw = dict(trace=True, tmpdir=tempfile.mkdtemp(prefix=prefix))
        return run_bass_kernel_spmd(ncX, in_maps, core_ids=list(range(8)),
                                    **kw)

    in_maps_A = []
    for c in range(8):
        in_maps_A.append(prep_A(points[(c // 2) % Bn], weights, c % 2, R))
    resA = _run(ncA, in_maps_A, "trA_")

    g1 = np.asarray(weights["g2d_0"])
    t1g = np.asarray(weights["t2d_0"])
    in_maps_B = []
    for b in range(Bn):
        e, o = resA.results[2 * b], resA.results[2 * b + 1]
        acc = np.concatenate([e["accx1o"], o["accx1o"]], 1)
        accsq = np.concatenate([e["accx1sqo"], o["accx1sqo"]], 1)
        s1, t1 = host_gn1(acc, accsq, g1, t1g, N * K)
        for half in range(2):
            x1s = np.ascontiguousarray(resA.results[2 * b + half]["x1o"])
            in_maps_B.append(prep_B(x1s, s1, t1, weights))
    resB = _run(ncB, in_maps_B, "trB_")

    in_maps_C = []
    for b in range(Bn):
        e, o = resB.results[2 * b], resB.results[2 * b + 1]
        s2, t2 = host_gn2([e["acczo"], o["acczo"]],
                          [e["accysqo"], o["accysqo"]], weights, N * K)
        ypool = np.concatenate([e["ypoolo"], o["ypoolo"]], 1)
        for hh in range(2):
            in_maps_C.append(prep_C(ypool, s2, t2, weights, hh))
    resC = _run(ncC, in_maps_C, "trC_")

    out = np.stack([
        np.concatenate([resC.results[2 * b]["out"],
                        resC.results[2 * b + 1]["out"]], 0)
        for b in range(Bn)], 0)
    if _trace:
        kernel.last_exec_A = resA.exec_time_ns
        kernel.last_exec_B = resB.exec_time_ns
        kernel.last_exec_C = resC.exec_time_ns
        kernel.last_exec_ns = ((resA.exec_time_ns or 0)
                               + (resB.exec_time_ns or 0)
                               + (resC.exec_time_ns or 0))
        kernel.trace_A = resA.instructions_and_trace
        kernel.trace_B = resB.instructions_and_trace
        kernel.trace_C = resC.instructions_and_trace
    return out.astype(np.float32)


# revision 8
# speedup vs baseline: 1.5663x; 1.0163x over previous
"""Trainium2 Bass kernel for nn_AbsoluteRelativePositionEmbedding.

Three SPMD launches over 8 NeuronCores:
  Launch A (8 cores): each core handles HALF of one batch's query rows.
    kNN(16) over all N=8192 candidates -> gather neighbor coords ->
    conv1(6->32) -> x1 + per-channel partial sums (for GroupNorm-1 stats).
  Host: merges the tiny GN1 partial sums per batch -> scale/shift vectors.
  Launch B (8 cores, row-split): each core processes its OWN phase-A rows:
    gn1+ELU -> conv2(32->64, block-diag float32r) -> max over K -> ypool
    half + GN2 partial sums. Elementwise work is packed 4 row-tiles deep so
    all 128 partitions are busy.
  Host: merges GN2 partials -> s2/t2.
  Launch C (8 cores): full batch per core, channel-split final conv:
    gn2+ELU -> conv3(64->128) -> gn3 (on device) + ELU -> conv4 half
    (128 of 256 out channels) -> gn4 (on device) + ELU -> out half.

The distance matmul uses a 3-level bf16 split (21 contraction rows) which
matches fp32 accuracy at 4x the PE rate. ELU(x)+1 = min(exp(x),1)+relu(x)
is used internally; the +1 is folded into the next conv's bias.
"""
import sys
sys.path.insert(0, '/opt/trn_rl_repo')

import numpy as np
import ml_dtypes

import concourse.bass as bass
import concourse.bacc as bacc
import concourse.mybir as mybir
import concourse.tile as tile
from concourse.bass import IndirectOffsetOnAxis
from concourse.bass_utils import run_bass_kernel_spmd
from concourse.masks import make_identity

F32 = mybir.dt.float32
F32R = mybir.dt.float32r
U32 = mybir.dt.uint32
BF = mybir.dt.bfloat16
AF = mybir.ActivationFunctionType
ALU = mybir.AluOpType
AX = mybir.AxisListType

K = 16
EPS = 1e-5
NEG = -1e30
GROUPS = 8


def _r(ap):
    """float32r view for full-rate fp32 matmuls."""
    return ap.bitcast(F32R)


def _elu1_affine_inplace(nc, pool, x, s, t):
    """x = elu(s*x + t) + 1 in place, using shared [128, f] scratch tags."""
    p, f = x.shape[0], x.shape[1]
    m = pool.tile([128, f], F32, tag="elu_scr_m")
    e = pool.tile([128, f], F32, tag="elu_scr_e")
    nc.scalar.activation(m[0:p, :], x, AF.Relu, scale=s, bias=t)
    nc.scalar.activation(e[0:p, :], x, AF.Exp, scale=s, bias=t)
    nc.vector.scalar_tensor_tensor(x, e[0:p, :], 1.0, m[0:p, :], ALU.min,
                                   ALU.add)


def _gn_coeffs(tc, nc, stats_pool, C, G, mean_c, ex2_c, bvec, gvec, tvec,
               indT, bindT, name, eps_ap):
    """Per-channel GN scale s / shift t so that gn(y+b) = s*y + t."""
    psp = tc.alloc_tile_pool(name=f"ps_{name}", bufs=1, space="PSUM")
    m1 = stats_pool.tile([C, 1], F32, tag=f"m1_{name}")
    m2 = stats_pool.tile([C, 1], F32, tag=f"m2_{name}")
    nc.vector.tensor_add(m1, mean_c, bvec)
    tmp = stats_pool.tile([C, 1], F32, tag=f"tmp_{name}")
    nc.vector.tensor_add(tmp, mean_c, m1)
    nc.vector.tensor_mul(tmp, tmp, bvec)
    nc.vector.tensor_add(m2, ex2_c, tmp)
    gm_ps = psp.tile([G, 1], F32, tag="gm_ps")
    gm_sb = stats_pool.tile([G, 1], F32, tag=f"gm_sb_{name}")
    bc1 = stats_pool.tile([C, 1], F32, tag=f"bc1_{name}")
    bc2 = stats_pool.tile([C, 1], F32, tag=f"bc2_{name}")
    bc_ps = psp.tile([C, 1], F32, tag="bc_ps")
    nc.tensor.matmul(gm_ps, indT, m1, start=True, stop=True)
    nc.scalar.copy(gm_sb, gm_ps)
    nc.tensor.matmul(bc_ps, bindT, gm_sb, start=True, stop=True)
    nc.scalar.copy(bc1, bc_ps)
    nc.tensor.matmul(gm_ps, indT, m2, start=True, stop=True)
    nc.scalar.copy(gm_sb, gm_ps)
    nc.tensor.matmul(bc_ps, bindT, gm_sb, start=True, stop=True)
    nc.scalar.copy(bc2, bc_ps)
    var = stats_pool.tile([C, 1], F32, tag=f"var_{name}")
    nc.vector.tensor_mul(tmp, bc1, bc1)
    nc.vector.tensor_sub(var, bc2, tmp)
    sd = stats_pool.tile([C, 1], F32, tag=f"sd_{name}")
    nc.scalar.activation(sd, var, AF.Sqrt, bias=eps_ap[0:C, :])
    s = stats_pool.tile([C, 1], F32, tag=f"s_{name}")
    nc.vector.reciprocal(s, sd)
    nc.vector.tensor_mul(s, s, gvec)
    t = stats_pool.tile([C, 1], F32, tag=f"t_{name}")
    nc.vector.tensor_sub(tmp, bc1, bvec)
    nc.vector.tensor_mul(tmp, s, tmp)
    nc.vector.tensor_sub(t, tvec, tmp)
    psp.release()
    return s, t


def build_phaseA(tc, outs, ins, N, R, ch=512):
    """kNN + gather + conv1 for R query rows against N candidates."""
    nc = tc.nc
    NTh = R // 128
    NCH = N // ch

    tbl_d = ins["tbl"]
    x1o_d = outs["x1o"]

    consts = tc.alloc_tile_pool(name="consts", bufs=1)
    persist = tc.alloc_tile_pool(name="persist", bufs=1)

    w1bTg = consts.tile([68, 32 * 16], F32, tag="w1bTg")
    nc.sync.dma_start(w1bTg, ins["w1bTg"])
    ident = consts.tile([128, 128], F32, tag="ident")
    make_identity(nc, ident)

    Q = persist.tile([4, R], F32, tag="Q")
    Qb = persist.tile([21, R], BF, tag="Qb")
    Cb = persist.tile([21, N], BF, tag="Cb")
    nc.sync.dma_start(Q, ins["pts"])
    nc.sync.dma_start(Qb, ins["qb"])
    nc.sync.dma_start(Cb, ins["cb"])

    accx1 = persist.tile([32, NTh * 4], F32, tag="accx1")
    accx1sq = persist.tile([32, NTh * 4], F32, tag="accx1sq")

    with tc.tile_pool(name="pa_sb", bufs=2) as pa, \
         tc.tile_pool(name="pa_nd", bufs=2) as pa_nd, \
         tc.tile_pool(name="pa_ps", bufs=2, space="PSUM") as psa, \
         tc.tile_pool(name="pa_ps1", bufs=2, space="PSUM") as psa1, \
         tc.tile_pool(name="pa_pst", bufs=2, space="PSUM") as psat:
        for t in range(NTh):
            r0 = t * 128
            ndsb = pa_nd.tile([128, N], F32, tag="ndsb")
            CW = min(1024, N)
            for cc in range(N // CW):
                nd_ps = psa.tile([128, CW], F32, tag="nd_ps")
                for h in range(CW // 512):
                    nc.tensor.matmul(
                        nd_ps[:, h * 512:(h + 1) * 512],
                        Qb[:, r0:r0 + 128],
                        Cb[:, cc * CW + h * 512:cc * CW + (h + 1) * 512],
                        start=True, stop=True)
                nc.scalar.copy(ndsb[:, cc * CW:(cc + 1) * CW], nd_ps)
            # top-8 per chunk
            cand = pa.tile([128, NCH * 8], F32, tag="cand")
            for cq in range(NCH):
                nc.vector.max(out=cand[:, cq * 8:(cq + 1) * 8],
                              in_=ndsb[:, cq * ch:(cq + 1) * ch])
            r1 = pa.tile([128, 8], F32, tag="r1")
            r2 = pa.tile([128, 8], F32, tag="r2")
            r3 = pa.tile([128, 8], F32, tag="r3")
            cand2 = pa.tile([128, NCH * 8], F32, tag="cand2")
            cand3 = pa.tile([128, NCH * 8], F32, tag="cand3")
            nc.vector.max(out=r1, in_=cand)
            nc.vector.match_replace(out=cand2, in_to_replace=r1,
                                    in_values=cand, imm_value=NEG)
            nc.vector.max(out=r2, in_=cand2)
            nc.vector.match_replace(out=cand3, in_to_replace=r2,
                                    in_values=cand2, imm_value=NEG)
            nc.vector.max(out=r3, in_=cand3)
            # winners: ranks 2..17 (rank 1 is self)
            wA = pa.tile([128, 8], F32, tag="wA")
            nc.vector.tensor_copy(wA[:, 0:7], r1[:, 1:8])
            nc.vector.tensor_copy(wA[:, 7:8], r3[:, 0:1])
            gidx = pa.tile([128, K], U32, tag="gidx")
            nc.vector.max_index(gidx[:, 0:8], wA, ndsb)
            nc.vector.max_index(gidx[:, 8:16], r2, ndsb)
            # gather neighbor coords (16B rows from the xyz0 table)
            gt = pa.tile([128, K * 4], F32, tag="gt")
            for k in range(K):
                nc.gpsimd.indirect_dma_start(
                    out=gt[:, 4 * k:4 * (k + 1)], out_offset=None,
                    in_=tbl_d,
                    in_offset=IndirectOffsetOnAxis(ap=gidx[:, k:k + 1],
                                                   axis=0))
            gtT_ps = psat.tile([64, 128], F32, tag="gtT_ps")
            nc.tensor.transpose(gtT_ps, gt, ident)
            rhs68 = pa.tile([68, 128], F32, tag="rhs68")
            nc.scalar.copy(rhs68[0:64, :], gtT_ps)
            nc.scalar.copy(rhs68[64:68, :], Q[:, r0:r0 + 128])
            # conv1 into (32, 128*K), position order (k, r)
            x1sb = pa.tile([32, 128 * K], F32, tag="x1sb")
            for q in range(K // 4):
                x1_ps = psa1.tile([32, 512], F32, tag="x1_ps")
                for j in range(4):
                    k = 4 * q + j
                    nc.tensor.matmul(x1_ps[:, j * 128:(j + 1) * 128],
                                     w1bTg[:, 32 * k:32 * (k + 1)], rhs68,
                                     start=True, stop=True)
                sl = slice(q * 512, (q + 1) * 512)
                nc.scalar.activation(
                    x1sb[:, sl], x1_ps, AF.Identity,
                    accum_out=accx1[:, 4 * t + q:4 * t + q + 1])
                sqsc = pa.tile([32, 512], F32, tag="sqsc")
                nc.scalar.activation(
                    sqsc, x1sb[:, sl], AF.Square,
                    accum_out=accx1sq[:, 4 * t + q:4 * t + q + 1])
            nc.sync.dma_start(x1o_d[t], x1sb)

    nc.sync.dma_start(outs["accx1o"], accx1)
    nc.sync.dma_start(outs["accx1sqo"], accx1sq)
    persist.release()
    consts.release()


def build_phaseB(tc, outs, ins, R):
    """Row-split: gn1+ELU -> conv2 (block-diag) -> max over K -> ypool half.

    x1 tiles are packed 4-deep on partitions: partition 32j+c holds channel
    c of row-tile 4t+j. conv2 runs as two block-diagonal (64->128) matmuls.
    Emits per-core GN2 partial sums (z sums, y^2 sums) for the host merge.
    """
    nc = tc.nc
    NP = R // 512                  # packed tiles (4 row-tiles each)

    x1s_d = ins["x1s"]
    yp_d = outs["ypoolo"]

    consts = tc.alloc_tile_pool(name="consts", bufs=1)
    persist = tc.alloc_tile_pool(name="persist", bufs=1)

    s1t = consts.tile([128, 1], F32, tag="s1t")
    t1t = consts.tile([128, 1], F32, tag="t1t")
    w2bdf = consts.tile([128, 128], F32, tag="w2bdf")
    w2bd = consts.tile([128, 128], F32R, tag="w2bd")
    nc.sync.dma_start(s1t, ins["s1t"])
    nc.sync.dma_start(t1t, ins["t1t"])
    nc.sync.dma_start(w2bdf, ins["w2bd"])
    nc.scalar.copy(w2bd, w2bdf)

    accz = persist.tile([128, NP], F32, tag="accz")
    accysq = persist.tile([128, 2 * NP], F32, tag="accysq")

    with tc.tile_pool(name="pb_sb", bufs=2) as pb, \
         tc.tile_pool(name="pb_scr", bufs=1) as pbs, \
         tc.tile_pool(name="pb_ps", bufs=2, space="PSUM") as psb:
        for t in range(NP):
            xp = pb.tile([128, 2048], F32, tag="xp")
            for j in range(4):
                nc.sync.dma_start(xp[32 * j:32 * (j + 1), :],
                                  x1s_d[4 * t + j])
            m_ = pb.tile([128, 2048], F32, tag="elu_m")
            e_ = pb.tile([128, 2048], F32, tag="elu_e")
            nc.scalar.activation(m_, xp, AF.Relu, scale=s1t, bias=t1t)
            nc.scalar.activation(e_, xp, AF.Exp, scale=s1t, bias=t1t)
            z = pb.tile([128, 2048], F32R, tag="z")
            # z = elu(gn1(x1)) + 1  (the +1 is folded into b2' on host)
            nc.vector.scalar_tensor_tensor(z, e_, 1.0, m_, ALU.min, ALU.add,
                                           accum_out=accz[:, t:t + 1])
            sqscr = pb.tile([128, 2048], F32, tag="sqscr")
            for h in range(2):     # row-tiles (4t+2h, 4t+2h+1)
                y_ps = psb.tile([128, 2048], F32, tag="y_ps")
                for q in range(4):
                    sl = slice(q * 512, (q + 1) * 512)
                    nc.tensor.matmul(y_ps[:, sl],
                                     w2bd[64 * h:64 * (h + 1), :],
                                     z[64 * h:64 * (h + 1), sl],
                                     start=True, stop=True)
                nc.scalar.activation(sqscr, y_ps, AF.Square,
                                     accum_out=accysq[:, 2 * t + h:2 * t + h + 1])
                # max over K (position order (k, r): fold k halves)
                p1 = pb.tile([128, 1024], F32, tag="p1")
                p2 = pb.tile([128, 512], F32, tag="p2")
                p3 = pb.tile([128, 256], F32, tag="p3")
                p4 = pb.tile([128, 128], F32, tag="p4")
                yh = pb.tile([128, 1024], F32, tag="yh")
                nc.scalar.copy(yh, y_ps[:, 1024:2048])
                nc.vector.tensor_tensor(p1, y_ps[:, 0:1024],
                                        yh, op=ALU.max)
                nc.vector.tensor_tensor(p2, p1[:, 0:512], p1[:, 512:1024],
                                        op=ALU.max)
                nc.vector.tensor_tensor(p3, p2[:, 0:256], p2[:, 256:512],
                                        op=ALU.max)
                nc.vector.tensor_tensor(p4, p3[:, 0:128], p3[:, 128:256],
                                        op=ALU.max)
                rt = 4 * t + 2 * h
                nc.sync.dma_start(yp_d[:, rt * 128:(rt + 1) * 128],
                                  p4[0:64, :])
                nc.sync.dma_start(yp_d[:, (rt + 1) * 128:(rt + 2) * 128],
                                  p4[64:128, :])

    nc.sync.dma_start(outs["acczo"], accz)
    nc.sync.dma_start(outs["accysqo"], accysq)
    persist.release()
    consts.release()


def _elu1_chunked(nc, pool, x, s, t, xin=None, nch=4):
    """x_out = elu(s*xin + t) + 1, chunked so scalar/vector pipeline."""
    p, f = x.shape[0], x.shape[1]
    if xin is None:
        xin = x.bitcast(F32) if x.dtype != F32 else x
    cw = f // nch
    m = pool.tile([128, f], F32, tag="elu_scr_m")
    e = pool.tile([128, f], F32, tag="elu_scr_e")
    for c in range(nch):
        sl = slice(c * cw, (c + 1) * cw)
        nc.scalar.activation(m[0:p, sl], xin[:, sl], AF.Relu, scale=s,
                             bias=t)
        nc.scalar.activation(e[0:p, sl], xin[:, sl], AF.Exp, scale=s,
                             bias=t)
        nc.vector.scalar_tensor_tensor(x[:, sl], e[0:p, sl], 1.0,
                                       m[0:p, sl], ALU.min, ALU.add)


def build_phaseC(tc, outs, ins, N):
    """Full batch: gn2+ELU -> conv3 -> gn3+ELU -> conv4 half -> gn4+ELU.

    ypool is packed 2-deep on partitions (partition 64h+c = channel c for
    column half h). conv4 computes only this core's 128 of 256 channels;
    its 4 GN groups are self-contained so gn4 runs fully on device.
    """
    nc = tc.nc
    NH = N // 2

    out_d = outs["out"]

    consts = tc.alloc_tile_pool(name="consts", bufs=1)
    stats_pool = tc.alloc_tile_pool(name="stats", bufs=1)
    persist = tc.alloc_tile_pool(name="persist", bufs=1)

    def load_const(name, shape):
        t = consts.tile(shape, F32, tag=name)
        nc.sync.dma_start(t, ins[name])
        return t

    s2t = load_const("s2t", [128, 1])
    t2t = load_const("t2t", [128, 1])
    w3T2f = load_const("w3T2", [128, 128])
    w4hf = load_const("w4h", [128, 128])
    w3T2 = consts.tile([128, 128], F32R, tag="w3T2r")
    w4h = consts.tile([128, 128], F32R, tag="w4hr")
    nc.scalar.copy(w3T2, w3T2f)
    nc.scalar.copy(w4h, w4hf)
    b3 = load_const("b3", [128, 1])
    g3 = load_const("g3", [128, 1])
    t3 = load_const("t3", [128, 1])
    b4 = load_const("b4", [128, 1])
    g4 = load_const("g4", [128, 1])
    t4 = load_const("t4", [128, 1])
    ind3T = load_const("indT_c128", [128, 8])
    bind3T = load_const("bindT_c128", [8, 128])
    ind4T = load_const("indT_c128h", [128, 4])
    bind4T = load_const("bindT_c128h", [4, 128])
    eps_t = consts.tile([128, 1], F32, tag="eps_t")
    nc.vector.memset(eps_t, EPS)

    with tc.tile_pool(name="pc_sb", bufs=1) as pc, \
         tc.tile_pool(name="pc_ps", bufs=2, space="PSUM") as psc:
        yp = pc.tile([128, NH], F32, tag="yp")
        nc.sync.dma_start(yp[0:64, :], ins["ypool"][:, 0:NH])
        nc.sync.dma_start(yp[64:128, :], ins["ypool"][:, NH:N])
        u = pc.tile([128, NH], F32R, tag="u")
        _elu1_chunked(nc, pc, u, s2t, t2t, xin=yp)

        NCH = NH // 1024
        vs = []
        accv = stats_pool.tile([128, 2 * NCH], F32, tag="accv")
        accvsq = stats_pool.tile([128, 2 * NCH], F32, tag="accvsq")
        sqscr = pc.tile([128, 1024], F32, tag="sqscr")
        for hh in range(2):
            v = pc.tile([128, NH], F32R, tag=f"v{hh}")
            for cch in range(NCH):
                sl = slice(cch * 1024, (cch + 1) * 1024)
                v_ps = psc.tile([128, 1024], F32, tag="mm_ps")
                for q in range(2):
                    sq_ = slice(cch * 1024 + q * 512,
                                cch * 1024 + (q + 1) * 512)
                    nc.tensor.matmul(v_ps[:, q * 512:(q + 1) * 512],
                                     w3T2[64 * hh:64 * (hh + 1), :],
                                     u[64 * hh:64 * (hh + 1), sq_],
                                     start=True, stop=True)
                ci = hh * NCH + cch
                nc.scalar.activation(v[:, sl], v_ps, AF.Identity,
                                     accum_out=accv[:, ci:ci + 1])
                nc.scalar.activation(sqscr, v.bitcast(F32)[:, sl], AF.Square,
                                     accum_out=accvsq[:, ci:ci + 1])
            vs.append(v)
        mean3 = stats_pool.tile([128, 1], F32, tag="mean3")
        ex23 = stats_pool.tile([128, 1], F32, tag="ex23")
        nc.vector.tensor_reduce(mean3, accv, AX.X, ALU.add)
        nc.vector.tensor_reduce(ex23, accvsq, AX.X, ALU.add)
        nc.vector.tensor_scalar_mul(mean3, mean3, 1.0 / N)
        nc.vector.tensor_scalar_mul(ex23, ex23, 1.0 / N)
        s3, t3c = _gn_coeffs(tc, nc, stats_pool, 128, GROUPS, mean3, ex23,
                             b3, g3, t3, ind3T, bind3T, name="gn3",
                             eps_ap=eps_t)
        for v in vs:
            _elu1_chunked(nc, pc, v, s3, t3c)

        os_ = []
        acco = stats_pool.tile([128, 2 * NCH], F32, tag="acco")
        accosq = stats_pool.tile([128, 2 * NCH], F32, tag="accosq")
        for hh in range(2):
            o = pc.tile([128, NH], F32, tag=f"o{hh}")
            for cch in range(NCH):
                sl = slice(cch * 1024, (cch + 1) * 1024)
                o_ps = psc.tile([128, 1024], F32, tag="mm_ps")
                for q in range(2):
                    sq_ = slice(cch * 1024 + q * 512,
                                cch * 1024 + (q + 1) * 512)
                    nc.tensor.matmul(o_ps[:, q * 512:(q + 1) * 512],
                                     w4h, vs[hh][:, sq_],
                                     start=True, stop=True)
                ci = hh * NCH + cch
                nc.scalar.activation(o[:, sl], o_ps, AF.Identity,
                                     accum_out=acco[:, ci:ci + 1])
                nc.scalar.activation(sqscr, o[:, sl], AF.Square,
                                     accum_out=accosq[:, ci:ci + 1])
            os_.append(o)
        mean4 = stats_pool.tile([128, 1], F32, tag="mean4")
        ex24 = stats_pool.tile([128, 1], F32, tag="ex24")
        nc.vector.tensor_reduce(mean4, acco, AX.X, ALU.add)
        nc.vector.tensor_reduce(ex24, accosq, AX.X, ALU.add)
        nc.vector.tensor_scalar_mul(mean4, mean4, 1.0 / N)
        nc.vector.tensor_scalar_mul(ex24, ex24, 1.0 / N)
        s4, t4c = _gn_coeffs(tc, nc, stats_pool, 128, 4, mean4, ex24,
                             b4, g4, t4, ind4T, bind4T, name="gn4",
                             eps_ap=eps_t)
        for hh in range(2):
            o = os_[hh]
            _elu1_chunked(nc, pc, o, s4, t4c)
            nc.vector.tensor_scalar_add(o, o, -1.0)
            nc.sync.dma_start(out_d[:, hh * NH:(hh + 1) * NH], o)

    persist.release()
    stats_pool.release()
    consts.release()


# ---------------- host-side prep ----------------

def _inds(Cc, Gg):
    gs = Cc // Gg
    indT = np.zeros((Cc, Gg), np.float32)
    bindT = np.zeros((Gg, Cc), np.float32)
    for g in range(Gg):
        indT[g * gs:(g + 1) * gs, g] = 1.0 / gs
        bindT[g, g * gs:(g + 1) * gs] = 1.0
    return indT, bindT


def _w1big(w1b, w1cT):
    big = np.zeros((68, 32 * 16), np.float32)
    for k in range(16):
        big[4 * k:4 * k + 3, 32 * k:32 * (k + 1)] = w1b.T
        big[64:68, 32 * k:32 * (k + 1)] = w1cT
    return big


def _split3(x):
    h = x.astype(ml_dtypes.bfloat16)
    r = x.astype(np.float32) - h.astype(np.float32)
    m = r.astype(ml_dtypes.bfloat16)
    l = (r - m.astype(np.float32)).astype(ml_dtypes.bfloat16)
    return h, m, l


def _qb_cb(points_b):
    p = points_b.astype(np.float64)
    sq = (p * p).sum(0)
    h, m, l = _split3(points_b.astype(np.float32))
    sh, sm, sl = _split3(sq.astype(np.float32))
    BFD = ml_dtypes.bfloat16
    ones = np.ones_like(h[0:1])
    qb = np.concatenate([h, h, m, m, h, l, ones, ones, ones], 0)
    cb = np.concatenate([(2 * h.astype(np.float32)).astype(BFD),
                         (2 * m.astype(np.float32)).astype(BFD),
                         (2 * h.astype(np.float32)).astype(BFD),
                         (2 * m.astype(np.float32)).astype(BFD),
                         (2 * l.astype(np.float32)).astype(BFD),
                         (2 * h.astype(np.float32)).astype(BFD),
                         -sh[None, :], -sm[None, :], -sl[None, :]], 0)
    return np.ascontiguousarray(qb), np.ascontiguousarray(cb)


def prep_A(points_b, weights, half, R):
    """Inputs for one phase-A core: query rows [half*R, (half+1)*R)."""
    N = points_b.shape[1]
    w1 = np.asarray(weights["w2d_0"])
    b1 = np.asarray(weights["b2d_0"])
    w1a, w1b = w1[:, 0:3], w1[:, 3:6]
    w1cT = np.concatenate([(w1a - w1b).T, b1[None, :]], 0).astype(np.float32)
    qb, cb = _qb_cb(points_b)
    sl = slice(half * R, (half + 1) * R)
    pts = points_b.astype(np.float32)
    ptsq = np.concatenate([pts[:, sl], np.ones((1, R), np.float32)], 0)
    tbl = np.concatenate([pts.T, np.zeros((N, 1), np.float32)], 1)
    return {
        "pts": np.ascontiguousarray(ptsq),
        "qb": np.ascontiguousarray(qb[:, sl]),
        "cb": cb,
        "tbl": np.ascontiguousarray(tbl),
        "w1bTg": np.ascontiguousarray(_w1big(w1b, w1cT)),
    }


def prep_B(x1s, s1, t1, weights):
    w2T = np.asarray(weights["w2d_1"]).T.astype(np.float32)   # (32, 64)
    w2bd = np.zeros((128, 128), np.float32)
    for rep in range(2):
        w2bd[64 * rep + 0:64 * rep + 32, 0:64] = w2T
        w2bd[64 * rep + 32:64 * rep + 64, 64:128] = w2T
    return {
        "x1s": x1s,
        "s1t": np.ascontiguousarray(np.tile(s1.reshape(-1), 4).reshape(-1, 1).astype(np.float32)),
        "t1t": np.ascontiguousarray(np.tile(t1.reshape(-1), 4).reshape(-1, 1).astype(np.float32)),
        "w2bd": np.ascontiguousarray(w2bd),
    }


def host_gn1(accx1, accx1sq, g1, t1g, NK):
    """Host-side GN1 scale/shift from merged per-channel partial sums."""
    mean1 = accx1.sum(1, dtype=np.float64) / NK
    ex2 = accx1sq.sum(1, dtype=np.float64) / NK
    gm = mean1.reshape(8, 4).mean(1)
    gex2 = ex2.reshape(8, 4).mean(1)
    var = gex2 - gm * gm
    s = np.asarray(g1, np.float64) / np.sqrt(var.repeat(4) + EPS)
    t = np.asarray(t1g, np.float64) - s * gm.repeat(4)
    return s.astype(np.float32), t.astype(np.float32)


def host_gn2(acczs, accysqs, weights, NK):
    """GN2 scale/shift from the two cores' partial sums of one batch."""
    w2 = np.asarray(weights["w2d_1"], np.float64)            # (64, 32)
    b2f = (np.asarray(weights["b2d_1"], np.float64)
           - w2.sum(1))                                       # +1 fold
    zsum = np.zeros(32, np.float64)
    eysq = np.zeros(64, np.float64)
    for accz, accysq in zip(acczs, accysqs):
        zsum += accz.astype(np.float64).reshape(4, 32, -1).sum((0, 2))
        eysq += accysq.astype(np.float64).reshape(2, 64, -1).sum((0, 2))
    ymean = (w2 @ zsum) / NK
    mean_t = ymean + b2f
    e2_t = eysq / NK + 2.0 * b2f * ymean + b2f * b2f
    gm = mean_t.reshape(8, 8).mean(1)
    ge2 = e2_t.reshape(8, 8).mean(1)
    var = ge2 - gm * gm
    g2 = np.asarray(weights["g2d_1"], np.float64)
    t2 = np.asarray(weights["t2d_1"], np.float64)
    s = g2 / np.sqrt(var.repeat(8) + EPS)
    # out = g*((y+b2f) - gm)/sd + t2 = s*y + (t2 - s*gm + s*b2f)
    t = t2 - s * gm.repeat(8) + s * b2f
    return s.astype(np.float32), t.astype(np.float32)


def prep_C(ypool, s2, t2, weights, hh):
    w3T = np.asarray(weights["w1d_0"]).T.astype(np.float32)   # (64, 128)
    w3T2 = np.concatenate([w3T, w3T], 0)                      # (128, 128)
    w4T = np.asarray(weights["w1d_1"]).T.astype(np.float32)   # (128, 256)
    b3f = (np.asarray(weights["b1d_0"])
           - np.asarray(weights["w1d_0"]).sum(1))
    b4f = (np.asarray(weights["b1d_1"])
           - np.asarray(weights["w1d_1"]).sum(1))
    sl4 = slice(hh * 128, (hh + 1) * 128)
    m = {
        "ypool": np.ascontiguousarray(ypool),
        "s2t": np.ascontiguousarray(np.tile(s2, 2).reshape(-1, 1).astype(np.float32)),
        "t2t": np.ascontiguousarray(np.tile(t2, 2).reshape(-1, 1).astype(np.float32)),
        "w3T2": np.ascontiguousarray(w3T2),
        "w4h": np.ascontiguousarray(w4T[:, sl4]),
        "b3": b3f.reshape(-1, 1).astype(np.float32),
        "g3": np.asarray(weights["g1d_0"]).reshape(-1, 1).astype(np.float32),
        "t3": np.asarray(weights["t1d_0"]).reshape(-1, 1).astype(np.float32),
        "b4": np.ascontiguousarray(b4f[sl4].reshape(-1, 1).astype(np.float32)),
        "g4": np.ascontiguousarray(np.asarray(weights["g1d_1"])[sl4].reshape(-1, 1).astype(np.float32)),
        "t4": np.ascontiguousarray(np.asarray(weights["t1d_1"])[sl4].reshape(-1, 1).astype(np.float32)),
    }
    for nm, (cc, gg) in (("c128", (128, 8)), ("c128h", (128, 4))):
        indT, bindT = _inds(cc, gg)
        m[f"indT_{nm}"] = indT
        m[f"bindT_{nm}"] = bindT
    return m


_CACHE = {}


def _build_A(N, R, ch=512):
    key = ("A", N, R, ch)
    if key in _CACHE:
        return _CACHE[key]
    nc = bacc.Bacc("TRN2", target_bir_lowering=False, debug=False,
                   num_devices=8)
    NTh = R // 128
    ins = {
        "pts": nc.dram_tensor("pts", [4, R], F32, kind="ExternalInput").ap(),
        "qb": nc.dram_tensor("qb", [21, R], BF, kind="ExternalInput").ap(),
        "cb": nc.dram_tensor("cb", [21, N], BF, kind="ExternalInput").ap(),
        "tbl": nc.dram_tensor("tbl", [N, 4], F32, kind="ExternalInput").ap(),
        "w1bTg": nc.dram_tensor("w1bTg", [68, 32 * 16], F32,
                                kind="ExternalInput").ap(),
    }
    outs = {
        "x1o": nc.dram_tensor("x1o", [NTh, 32, 128 * K], F32,
                              kind="ExternalOutput").ap(),
        "accx1o": nc.dram_tensor("accx1o", [32, NTh * 4], F32,
                                 kind="ExternalOutput").ap(),
        "accx1sqo": nc.dram_tensor("accx1sqo", [32, NTh * 4], F32,
                                   kind="ExternalOutput").ap(),
    }
    with tile.TileContext(nc) as tc:
        build_phaseA(tc, outs, ins, N, R, ch)
    nc.compile()
    _CACHE[key] = nc
    return nc


def _build_B(R):
    key = ("B", R)
    if key in _CACHE:
        return _CACHE[key]
    nc = bacc.Bacc("TRN2", target_bir_lowering=False, debug=False,
                   num_devices=8)
    NTh = R // 128
    NP = R // 512
    ins = {
        "x1s": nc.dram_tensor("x1s", [NTh, 32, 128 * K], F32,
                              kind="ExternalInput").ap(),
        "s1t": nc.dram_tensor("s1t", [128, 1], F32, kind="ExternalInput").ap(),
        "t1t": nc.dram_tensor("t1t", [128, 1], F32, kind="ExternalInput").ap(),
        "w2bd": nc.dram_tensor("w2bd", [128, 128], F32,
                               kind="ExternalInput").ap(),
    }
    outs = {
        "ypoolo": nc.dram_tensor("ypoolo", [64, R], F32,
                                 kind="ExternalOutput").ap(),
        "acczo": nc.dram_tensor("acczo", [128, NP], F32,
                                kind="ExternalOutput").ap(),
        "accysqo": nc.dram_tensor("accysqo", [128, 2 * NP], F32,
                                  kind="ExternalOutput").ap(),
    }
    with tile.TileContext(nc) as tc:
        build_phaseB(tc, outs, ins, R)
    nc.compile()
    _CACHE[key] = nc
    return nc


def _build_C(N):
    key = ("C", N)
    if key in _CACHE:
        return _CACHE[key]
    nc = bacc.Bacc("TRN2", target_bir_lowering=False, debug=False,
                   num_devices=8)
    ins = {"ypool": nc.dram_tensor("ypool", [64, N], F32,
                                   kind="ExternalInput").ap()}
    specs = [("s2t", [128, 1]), ("t2t", [128, 1]), ("w3T2", [128, 128]),
             ("w4h", [128, 128]), ("b3", [128, 1]), ("g3", [128, 1]),
             ("t3", [128, 1]), ("b4", [128, 1]), ("g4", [128, 1]),
             ("t4", [128, 1]),
             ("indT_c128", [128, 8]), ("bindT_c128", [8, 128]),
             ("indT_c128h", [128, 4]), ("bindT_c128h", [4, 128])]
    for nm, shape in specs:
        ins[nm] = nc.dram_tensor(nm, shape, F32, kind="ExternalInput").ap()
    outs = {"out": nc.dram_tensor("out", [128, N], F32,
                                  kind="ExternalOutput").ap()}
    with tile.TileContext(nc) as tc:
        build_phaseC(tc, outs, ins, N)
    nc.compile()
    _CACHE[key] = nc
    return nc


def kernel(points, _trace=False, **weights):
    points = np.asarray(points)
    Bn, _, N = points.shape
    R = N // 2
    ncA = _build_A(N, R)
    ncB = _build_B(R)
    ncC = _build_C(N)

    if _trace:
        import tempfile

    def _run(ncX, in_maps, prefix):
        kw = {}
        if _trace:
            kw = dict(trace=True, tmpdir=tempfile.mkdtemp(prefix=prefix))
        return run_bass_kernel_spmd(ncX, in_maps, core_ids=list(range(8)),
                                    **kw)

    in_maps_A = []
    for c in range(8):
        in_maps_A.append(prep_A(points[(c // 2) % Bn], weights, c % 2, R))
    resA = _run(ncA, in_maps_A, "trA_")

    g1 = np.asarray(weights["g2d_0"])
    t1g = np.asarray(weights["t2d_0"])
    in_maps_B = []
    for b in range(Bn):
        e, o = resA.results[2 * b], resA.results[2 * b + 1]
        acc = np.concatenate([e["accx1o"], o["accx1o"]], 1)
        accsq = np.concatenate([e["accx1sqo"], o["accx1sqo"]], 1)
        s1, t1 = host_gn1(acc, accsq, g1, t1g, N * K)
        for half in range(2):
            x1s = np.ascontiguousarray(resA.results[2 * b + half]["x1o"])
            in_maps_B.append(prep_B(x1s, s1, t1, weights))
    resB = _run(ncB, in_maps_B, "trB_")

    in_maps_C = []
    for b in range(Bn):
        e, o = resB.results[2 * b], resB.results[2 * b + 1]
        s2, t2 = host_gn2([e["acczo"], o["acczo"]],
                          [e["accysqo"], o["accysqo"]], weights, N * K)
        ypool = np.concatenate([e["ypoolo"], o["ypoolo"]], 1)
        for hh in range(2):
            in_maps_C.append(prep_C(ypool, s2, t2, weights, hh))
    resC = _run(ncC, in_maps_C, "trC_")

    out = np.stack([
        np.concatenate([resC.results[2 * b]["out"],
                        resC.results[2 * b + 1]["out"]], 0)
        for b in range(Bn)], 0)
    if _trace:
        kernel.last_exec_A = resA.exec_time_ns
        kernel.last_exec_B = resB.exec_time_ns
        kernel.last_exec_C = resC.exec_time_ns
        kernel.last_exec_ns = ((resA.exec_time_ns or 0)
                               + (resB.exec_time_ns or 0)
                               + (resC.exec_time_ns or 0))
        kernel.trace_A = resA.instructions_and_trace
        kernel.trace_B = resB.instructions_and_trace
        kernel.trace_C = resC.instructions_and_trace
    return out.astype(np.float32)


# revision 9
# speedup vs baseline: 1.5772x; 1.0070x over previous
"""Trainium2 Bass kernel for nn_AbsoluteRelativePositionEmbedding.

Three SPMD launches over 8 NeuronCores:
  Launch A (8 cores): each core handles HALF of one batch's query rows.
    kNN(16) over all N=8192 candidates -> gather neighbor coords ->
    conv1(6->32) -> x1 + per-channel partial sums (for GroupNorm-1 stats).
  Host: merges the tiny GN1 partial sums per batch -> scale/shift vectors.
  Launch B (8 cores, row-split): each core processes its OWN phase-A rows:
    gn1+ELU -> conv2(32->64, block-diag float32r) -> max over K -> ypool
    half + GN2 partial sums. Elementwise work is packed 4 row-tiles deep so
    all 128 partitions are busy.
  Host: merges GN2 partials -> s2/t2.
  Launch C (8 cores): full batch per core, channel-split final conv:
    gn2+ELU -> conv3(64->128) -> gn3 (on device) + ELU -> conv4 half
    (128 of 256 out channels) -> gn4 (on device) + ELU -> out half.

The distance matmul uses a 3-level bf16 split (21 contraction rows) which
matches fp32 accuracy at 4x the PE rate. ELU(x)+1 = min(exp(x),1)+relu(x)
is used internally; the +1 is folded into the next conv's bias.
"""
import sys
sys.path.insert(0, '/opt/trn_rl_repo')

import numpy as np
import ml_dtypes

import concourse.bass as bass
import concourse.bacc as bacc
import concourse.mybir as mybir
import concourse.tile as tile
from concourse.bass import IndirectOffsetOnAxis
from concourse.bass_utils import run_bass_kernel_spmd
from concourse.masks import make_identity

F32 = mybir.dt.float32
F32R = mybir.dt.float32r
U32 = mybir.dt.uint32
BF = mybir.dt.bfloat16
AF = mybir.ActivationFunctionType
ALU = mybir.AluOpType
AX = mybir.AxisListType

K = 16
EPS = 1e-5
NEG = -1e30
GROUPS = 8


def _r(ap):
    """float32r view for full-rate fp32 matmuls."""
    return ap.bitcast(F32R)


def _elu1_affine_inplace(nc, pool, x, s, t):
    """x = elu(s*x + t) + 1 in place, using shared [128, f] scratch tags."""
    p, f = x.shape[0], x.shape[1]
    m = pool.tile([128, f], F32, tag="elu_scr_m")
    e = pool.tile([128, f], F32, tag="elu_scr_e")
    nc.scalar.activation(m[0:p, :], x, AF.Relu, scale=s, bias=t)
    nc.scalar.activation(e[0:p, :], x, AF.Exp, scale=s, bias=t)
    nc.vector.scalar_tensor_tensor(x, e[0:p, :], 1.0, m[0:p, :], ALU.min,
                                   ALU.add)


def _gn_coeffs(tc, nc, stats_pool, C, G, mean_c, ex2_c, bvec, gvec, tvec,
               indT, bindT, name, eps_ap):
    """Per-channel GN scale s / shift t so that gn(y+b) = s*y + t."""
    psp = tc.alloc_tile_pool(name=f"ps_{name}", bufs=1, space="PSUM")
    m1 = stats_pool.tile([C, 1], F32, tag=f"m1_{name}")
    m2 = stats_pool.tile([C, 1], F32, tag=f"m2_{name}")
    nc.vector.tensor_add(m1, mean_c, bvec)
    tmp = stats_pool.tile([C, 1], F32, tag=f"tmp_{name}")
    nc.vector.tensor_add(tmp, mean_c, m1)
    nc.vector.tensor_mul(tmp, tmp, bvec)
    nc.vector.tensor_add(m2, ex2_c, tmp)
    gm_ps = psp.tile([G, 1], F32, tag="gm_ps")
    gm_sb = stats_pool.tile([G, 1], F32, tag=f"gm_sb_{name}")
    bc1 = stats_pool.tile([C, 1], F32, tag=f"bc1_{name}")
    bc2 = stats_pool.tile([C, 1], F32, tag=f"bc2_{name}")
    bc_ps = psp.tile([C, 1], F32, tag="bc_ps")
    nc.tensor.matmul(gm_ps, indT, m1, start=True, stop=True)
    nc.scalar.copy(gm_sb, gm_ps)
    nc.tensor.matmul(bc_ps, bindT, gm_sb, start=True, stop=True)
    nc.scalar.copy(bc1, bc_ps)
    nc.tensor.matmul(gm_ps, indT, m2, start=True, stop=True)
    nc.scalar.copy(gm_sb, gm_ps)
    nc.tensor.matmul(bc_ps, bindT, gm_sb, start=True, stop=True)
    nc.scalar.copy(bc2, bc_ps)
    var = stats_pool.tile([C, 1], F32, tag=f"var_{name}")
    nc.vector.tensor_mul(tmp, bc1, bc1)
    nc.vector.tensor_sub(var, bc2, tmp)
    sd = stats_pool.tile([C, 1], F32, tag=f"sd_{name}")
    nc.scalar.activation(sd, var, AF.Sqrt, bias=eps_ap[0:C, :])
    s = stats_pool.tile([C, 1], F32, tag=f"s_{name}")
    nc.vector.reciprocal(s, sd)
    nc.vector.tensor_mul(s, s, gvec)
    t = stats_pool.tile([C, 1], F32, tag=f"t_{name}")
    nc.vector.tensor_sub(tmp, bc1, bvec)
    nc.vector.tensor_mul(tmp, s, tmp)
    nc.vector.tensor_sub(t, tvec, tmp)
    psp.release()
    return s, t


def build_phaseA(tc, outs, ins, N, R, ch=512):
    """kNN + gather + conv1 for R query rows against N candidates."""
    nc = tc.nc
    NTh = R // 128
    NCH = N // ch

    tbl_d = ins["tbl"]
    x1o_d = outs["x1o"]

    consts = tc.alloc_tile_pool(name="consts", bufs=1)
    persist = tc.alloc_tile_pool(name="persist", bufs=1)

    w1bTg = consts.tile([68, 32 * 16], F32, tag="w1bTg")
    nc.sync.dma_start(w1bTg, ins["w1bTg"])
    ident = consts.tile([128, 128], F32, tag="ident")
    make_identity(nc, ident)

    Q = persist.tile([4, R], F32, tag="Q")
    Qb = persist.tile([21, R], BF, tag="Qb")
    Cb = persist.tile([21, N], BF, tag="Cb")
    nc.sync.dma_start(Q, ins["pts"])
    nc.sync.dma_start(Qb, ins["qb"])
    nc.sync.dma_start(Cb, ins["cb"])

    accx1 = persist.tile([32, NTh * 4], F32, tag="accx1")
    accx1sq = persist.tile([32, NTh * 4], F32, tag="accx1sq")

    with tc.tile_pool(name="pa_sb", bufs=2) as pa, \
         tc.tile_pool(name="pa_nd", bufs=2) as pa_nd, \
         tc.tile_pool(name="pa_ps", bufs=2, space="PSUM") as psa, \
         tc.tile_pool(name="pa_ps1", bufs=2, space="PSUM") as psa1, \
         tc.tile_pool(name="pa_pst", bufs=2, space="PSUM") as psat:
        for t in range(NTh):
            r0 = t * 128
            ndsb = pa_nd.tile([128, N], F32, tag="ndsb")
            CW = min(1024, N)
            for cc in range(N // CW):
                nd_ps = psa.tile([128, CW], F32, tag="nd_ps")
                for h in range(CW // 512):
                    nc.tensor.matmul(
                        nd_ps[:, h * 512:(h + 1) * 512],
                        Qb[:, r0:r0 + 128],
                        Cb[:, cc * CW + h * 512:cc * CW + (h + 1) * 512],
                        start=True, stop=True)
                nc.scalar.copy(ndsb[:, cc * CW:(cc + 1) * CW], nd_ps)
            # top-8 per chunk
            cand = pa.tile([128, NCH * 8], F32, tag="cand")
            for cq in range(NCH):
                nc.vector.max(out=cand[:, cq * 8:(cq + 1) * 8],
                              in_=ndsb[:, cq * ch:(cq + 1) * ch])
            r1 = pa.tile([128, 8], F32, tag="r1")
            r2 = pa.tile([128, 8], F32, tag="r2")
            r3 = pa.tile([128, 8], F32, tag="r3")
            cand2 = pa.tile([128, NCH * 8], F32, tag="cand2")
            cand3 = pa.tile([128, NCH * 8], F32, tag="cand3")
            nc.vector.max(out=r1, in_=cand)
            nc.vector.match_replace(out=cand2, in_to_replace=r1,
                                    in_values=cand, imm_value=NEG)
            nc.vector.max(out=r2, in_=cand2)
            nc.vector.match_replace(out=cand3, in_to_replace=r2,
                                    in_values=cand2, imm_value=NEG)
            nc.vector.max(out=r3, in_=cand3)
            # winners: ranks 2..17 (rank 1 is self)
            wA = pa.tile([128, 8], F32, tag="wA")
            nc.vector.tensor_copy(wA[:, 0:7], r1[:, 1:8])
            nc.vector.tensor_copy(wA[:, 7:8], r3[:, 0:1])
            gidx = pa.tile([128, K], U32, tag="gidx")
            nc.vector.max_index(gidx[:, 0:8], wA, ndsb)
            nc.vector.max_index(gidx[:, 8:16], r2, ndsb)
            # gather neighbor coords (16B rows from the xyz0 table)
            gt = pa.tile([128, K * 4], F32, tag="gt")
            for k in range(K):
                nc.gpsimd.indirect_dma_start(
                    out=gt[:, 4 * k:4 * (k + 1)], out_offset=None,
                    in_=tbl_d,
                    in_offset=IndirectOffsetOnAxis(ap=gidx[:, k:k + 1],
                                                   axis=0))
            gtT_ps = psat.tile([64, 128], F32, tag="gtT_ps")
            nc.tensor.transpose(gtT_ps, gt, ident)
            rhs68 = pa.tile([68, 128], F32, tag="rhs68")
            nc.scalar.copy(rhs68[0:64, :], gtT_ps)
            nc.scalar.copy(rhs68[64:68, :], Q[:, r0:r0 + 128])
            # conv1 into (32, 128*K), position order (k, r)
            x1sb = pa.tile([32, 128 * K], F32, tag="x1sb")
            for q in range(K // 4):
                x1_ps = psa1.tile([32, 512], F32, tag="x1_ps")
                for j in range(4):
                    k = 4 * q + j
                    nc.tensor.matmul(x1_ps[:, j * 128:(j + 1) * 128],
                                     w1bTg[:, 32 * k:32 * (k + 1)], rhs68,
                                     start=True, stop=True)
                sl = slice(q * 512, (q + 1) * 512)
                nc.scalar.activation(
                    x1sb[:, sl], x1_ps, AF.Identity,
                    accum_out=accx1[:, 4 * t + q:4 * t + q + 1])
                sqsc = pa.tile([32, 512], F32, tag="sqsc")
                nc.scalar.activation(
                    sqsc, x1sb[:, sl], AF.Square,
                    accum_out=accx1sq[:, 4 * t + q:4 * t + q + 1])
            nc.sync.dma_start(x1o_d[t], x1sb)

    nc.sync.dma_start(outs["accx1o"], accx1)
    nc.sync.dma_start(outs["accx1sqo"], accx1sq)
    persist.release()
    consts.release()


def build_phaseB(tc, outs, ins, R):
    """Row-split: gn1+ELU -> conv2 (block-diag) -> max over K -> ypool half.

    x1 tiles are packed 4-deep on partitions: partition 32j+c holds channel
    c of row-tile 4t+j. conv2 runs as two block-diagonal (64->128) matmuls.
    Emits per-core GN2 partial sums (z sums, y^2 sums) for the host merge.
    """
    nc = tc.nc
    NP = R // 512                  # packed tiles (4 row-tiles each)

    x1s_d = ins["x1s"]
    yp_d = outs["ypoolo"]

    consts = tc.alloc_tile_pool(name="consts", bufs=1)
    persist = tc.alloc_tile_pool(name="persist", bufs=1)

    s1t = consts.tile([128, 1], F32, tag="s1t")
    t1t = consts.tile([128, 1], F32, tag="t1t")
    w2bdf = consts.tile([128, 128], F32, tag="w2bdf")
    w2bd = consts.tile([128, 128], F32R, tag="w2bd")
    nc.sync.dma_start(s1t, ins["s1t"])
    nc.sync.dma_start(t1t, ins["t1t"])
    nc.sync.dma_start(w2bdf, ins["w2bd"])
    nc.scalar.copy(w2bd, w2bdf)

    accz = persist.tile([128, 2 * NP], F32, tag="accz")
    accysq = persist.tile([128, 2 * NP], F32, tag="accysq")

    with tc.tile_pool(name="pb_sb", bufs=2) as pb, \
         tc.tile_pool(name="pb_scr", bufs=1) as pbs, \
         tc.tile_pool(name="pb_ps", bufs=2, space="PSUM") as psb:
        for t in range(NP):
            xp = pb.tile([128, 2048], F32, tag="xp")
            for j in range(4):
                nc.sync.dma_start(xp[32 * j:32 * (j + 1), :],
                                  x1s_d[4 * t + j])
            m_ = pb.tile([128, 2048], F32, tag="elu_m")
            e_ = pb.tile([128, 2048], F32, tag="elu_e")
            z = pb.tile([128, 2048], F32R, tag="z")
            # z = elu(gn1(x1)) + 1  (the +1 is folded into b2' on host)
            for q in range(2):
                sl = slice(q * 1024, (q + 1) * 1024)
                nc.scalar.activation(m_[:, sl], xp[:, sl], AF.Relu,
                                     scale=s1t, bias=t1t)
                nc.scalar.activation(e_[:, sl], xp[:, sl], AF.Exp,
                                     scale=s1t, bias=t1t)
                nc.vector.scalar_tensor_tensor(
                    z[:, sl], e_[:, sl], 1.0, m_[:, sl], ALU.min, ALU.add,
                    accum_out=accz[:, 2 * t + q:2 * t + q + 1])
            sqscr = pb.tile([128, 2048], F32, tag="sqscr")
            for h in range(2):     # row-tiles (4t+2h, 4t+2h+1)
                y_ps = psb.tile([128, 2048], F32, tag="y_ps")
                for q in range(4):
                    sl = slice(q * 512, (q + 1) * 512)
                    nc.tensor.matmul(y_ps[:, sl],
                                     w2bd[64 * h:64 * (h + 1), :],
                                     z[64 * h:64 * (h + 1), sl],
                                     start=True, stop=True)
                nc.scalar.activation(sqscr, y_ps, AF.Square,
                                     accum_out=accysq[:, 2 * t + h:2 * t + h + 1])
                # max over K (position order (k, r): fold k halves)
                p1 = pb.tile([128, 1024], F32, tag="p1")
                p2 = pb.tile([128, 512], F32, tag="p2")
                p3 = pb.tile([128, 256], F32, tag="p3")
                p4 = pb.tile([128, 128], F32, tag="p4")
                yh = pb.tile([128, 1024], F32, tag="yh")
                nc.scalar.copy(yh, y_ps[:, 1024:2048])
                nc.vector.tensor_tensor(p1, y_ps[:, 0:1024],
                                        yh, op=ALU.max)
                nc.vector.tensor_tensor(p2, p1[:, 0:512], p1[:, 512:1024],
                                        op=ALU.max)
                nc.vector.tensor_tensor(p3, p2[:, 0:256], p2[:, 256:512],
                                        op=ALU.max)
                nc.vector.tensor_tensor(p4, p3[:, 0:128], p3[:, 128:256],
                                        op=ALU.max)
                rt = 4 * t + 2 * h
                nc.sync.dma_start(yp_d[:, rt * 128:(rt + 1) * 128],
                                  p4[0:64, :])
                nc.sync.dma_start(yp_d[:, (rt + 1) * 128:(rt + 2) * 128],
                                  p4[64:128, :])

    nc.sync.dma_start(outs["acczo"], accz)
    nc.sync.dma_start(outs["accysqo"], accysq)
    persist.release()
    consts.release()


def _elu1_chunked(nc, pool, x, s, t, xin=None, nch=4):
    """x_out = elu(s*xin + t) + 1, chunked so scalar/vector pipeline."""
    p, f = x.shape[0], x.shape[1]
    if xin is None:
        xin = x.bitcast(F32) if x.dtype != F32 else x
    cw = f // nch
    m = pool.tile([128, f], F32, tag="elu_scr_m")
    e = pool.tile([128, f], F32, tag="elu_scr_e")
    for c in range(nch):
        sl = slice(c * cw, (c + 1) * cw)
        nc.scalar.activation(m[0:p, sl], xin[:, sl], AF.Relu, scale=s,
                             bias=t)
        nc.scalar.activation(e[0:p, sl], xin[:, sl], AF.Exp, scale=s,
                             bias=t)
        nc.vector.scalar_tensor_tensor(x[:, sl], e[0:p, sl], 1.0,
                                       m[0:p, sl], ALU.min, ALU.add)


def build_phaseC(tc, outs, ins, N):
    """Full batch: gn2+ELU -> conv3 -> gn3+ELU -> conv4 half -> gn4+ELU.

    ypool is packed 2-deep on partitions (partition 64h+c = channel c for
    column half h). conv4 computes only this core's 128 of 256 channels;
    its 4 GN groups are self-contained so gn4 runs fully on device.
    """
    nc = tc.nc
    NH = N // 2

    out_d = outs["out"]

    consts = tc.alloc_tile_pool(name="consts", bufs=1)
    stats_pool = tc.alloc_tile_pool(name="stats", bufs=1)
    persist = tc.alloc_tile_pool(name="persist", bufs=1)

    def load_const(name, shape):
        t = consts.tile(shape, F32, tag=name)
        nc.sync.dma_start(t, ins[name])
        return t

    s2t = load_const("s2t", [128, 1])
    t2t = load_const("t2t", [128, 1])
    w3T2f = load_const("w3T2", [128, 128])
    w4hf = load_const("w4h", [128, 128])
    w3T2 = consts.tile([128, 128], F32R, tag="w3T2r")
    w4h = consts.tile([128, 128], F32R, tag="w4hr")
    nc.scalar.copy(w3T2, w3T2f)
    nc.scalar.copy(w4h, w4hf)
    b3 = load_const("b3", [128, 1])
    g3 = load_const("g3", [128, 1])
    t3 = load_const("t3", [128, 1])
    b4 = load_const("b4", [128, 1])
    g4 = load_const("g4", [128, 1])
    t4 = load_const("t4", [128, 1])
    ind3T = load_const("indT_c128", [128, 8])
    bind3T = load_const("bindT_c128", [8, 128])
    ind4T = load_const("indT_c128h", [128, 4])
    bind4T = load_const("bindT_c128h", [4, 128])
    eps_t = consts.tile([128, 1], F32, tag="eps_t")
    nc.vector.memset(eps_t, EPS)

    with tc.tile_pool(name="pc_sb", bufs=1) as pc, \
         tc.tile_pool(name="pc_ps", bufs=2, space="PSUM") as psc:
        yp = pc.tile([128, NH], F32, tag="yp")
        nc.sync.dma_start(yp[0:64, :], ins["ypool"][:, 0:NH])
        nc.sync.dma_start(yp[64:128, :], ins["ypool"][:, NH:N])
        u = pc.tile([128, NH], F32R, tag="u")
        _elu1_chunked(nc, pc, u, s2t, t2t, xin=yp)

        NCH = NH // 1024
        vs = []
        accv = stats_pool.tile([128, 2 * NCH], F32, tag="accv")
        accvsq = stats_pool.tile([128, 2 * NCH], F32, tag="accvsq")
        sqscr = pc.tile([128, 1024], F32, tag="sqscr")
        for hh in range(2):
            v = pc.tile([128, NH], F32R, tag=f"v{hh}")
            for cch in range(NCH):
                sl = slice(cch * 1024, (cch + 1) * 1024)
                v_ps = psc.tile([128, 1024], F32, tag="mm_ps")
                for q in range(2):
                    sq_ = slice(cch * 1024 + q * 512,
                                cch * 1024 + (q + 1) * 512)
                    nc.tensor.matmul(v_ps[:, q * 512:(q + 1) * 512],
                                     w3T2[64 * hh:64 * (hh + 1), :],
                                     u[64 * hh:64 * (hh + 1), sq_],
                                     start=True, stop=True)
                ci = hh * NCH + cch
                nc.scalar.activation(v[:, sl], v_ps, AF.Identity,
                                     accum_out=accv[:, ci:ci + 1])
                nc.scalar.activation(sqscr, v.bitcast(F32)[:, sl], AF.Square,
                                     accum_out=accvsq[:, ci:ci + 1])
            vs.append(v)
        mean3 = stats_pool.tile([128, 1], F32, tag="mean3")
        ex23 = stats_pool.tile([128, 1], F32, tag="ex23")
        nc.vector.tensor_reduce(mean3, accv, AX.X, ALU.add)
        nc.vector.tensor_reduce(ex23, accvsq, AX.X, ALU.add)
        nc.vector.tensor_scalar_mul(mean3, mean3, 1.0 / N)
        nc.vector.tensor_scalar_mul(ex23, ex23, 1.0 / N)
        s3, t3c = _gn_coeffs(tc, nc, stats_pool, 128, GROUPS, mean3, ex23,
                             b3, g3, t3, ind3T, bind3T, name="gn3",
                             eps_ap=eps_t)
        for v in vs:
            _elu1_chunked(nc, pc, v, s3, t3c)

        os_ = []
        acco = stats_pool.tile([128, 2 * NCH], F32, tag="acco")
        accosq = stats_pool.tile([128, 2 * NCH], F32, tag="accosq")
        for hh in range(2):
            o = pc.tile([128, NH], F32, tag=f"o{hh}")
            for cch in range(NCH):
                sl = slice(cch * 1024, (cch + 1) * 1024)
                o_ps = psc.tile([128, 1024], F32, tag="mm_ps")
                for q in range(2):
                    sq_ = slice(cch * 1024 + q * 512,
                                cch * 1024 + (q + 1) * 512)
                    nc.tensor.matmul(o_ps[:, q * 512:(q + 1) * 512],
                                     w4h, vs[hh][:, sq_],
                                     start=True, stop=True)
                ci = hh * NCH + cch
                nc.scalar.activation(o[:, sl], o_ps, AF.Identity,
                                     accum_out=acco[:, ci:ci + 1])
                nc.scalar.activation(sqscr, o[:, sl], AF.Square,
                                     accum_out=accosq[:, ci:ci + 1])
            os_.append(o)
        mean4 = stats_pool.tile([128, 1], F32, tag="mean4")
        ex24 = stats_pool.tile([128, 1], F32, tag="ex24")
        nc.vector.tensor_reduce(mean4, acco, AX.X, ALU.add)
        nc.vector.tensor_reduce(ex24, accosq, AX.X, ALU.add)
        nc.vector.tensor_scalar_mul(mean4, mean4, 1.0 / N)
        nc.vector.tensor_scalar_mul(ex24, ex24, 1.0 / N)
        s4, t4c = _gn_coeffs(tc, nc, stats_pool, 128, 4, mean4, ex24,
                             b4, g4, t4, ind4T, bind4T, name="gn4",
                             eps_ap=eps_t)
        for hh in range(2):
            o = os_[hh]
            _elu1_chunked(nc, pc, o, s4, t4c)
            nc.vector.tensor_scalar_add(o, o, -1.0)
            nc.sync.dma_start(out_d[:, hh * NH:(hh + 1) * NH], o)

    persist.release()
    stats_pool.release()
    consts.release()


# ---------------- host-side prep ----------------

def _inds(Cc, Gg):
    gs = Cc // Gg
    indT = np.zeros((Cc, Gg), np.float32)
    bindT = np.zeros((Gg, Cc), np.float32)
    for g in range(Gg):
        indT[g * gs:(g + 1) * gs, g] = 1.0 / gs
        bindT[g, g * gs:(g + 1) * gs] = 1.0
    return indT, bindT


def _w1big(w1b, w1cT):
    big = np.zeros((68, 32 * 16), np.float32)
    for k in range(16):
        big[4 * k:4 * k + 3, 32 * k:32 * (k + 1)] = w1b.T
        big[64:68, 32 * k:32 * (k + 1)] = w1cT
    return big


def _split3(x):
    h = x.astype(ml_dtypes.bfloat16)
    r = x.astype(np.float32) - h.astype(np.float32)
    m = r.astype(ml_dtypes.bfloat16)
    l = (r - m.astype(np.float32)).astype(ml_dtypes.bfloat16)
    return h, m, l


def _qb_cb(points_b):
    p = points_b.astype(np.float64)
    sq = (p * p).sum(0)
    h, m, l = _split3(points_b.astype(np.float32))
    sh, sm, sl = _split3(sq.astype(np.float32))
    BFD = ml_dtypes.bfloat16
    ones = np.ones_like(h[0:1])
    qb = np.concatenate([h, h, m, m, h, l, ones, ones, ones], 0)
    cb = np.concatenate([(2 * h.astype(np.float32)).astype(BFD),
                         (2 * m.astype(np.float32)).astype(BFD),
                         (2 * h.astype(np.float32)).astype(BFD),
                         (2 * m.astype(np.float32)).astype(BFD),
                         (2 * l.astype(np.float32)).astype(BFD),
                         (2 * h.astype(np.float32)).astype(BFD),
                         -sh[None, :], -sm[None, :], -sl[None, :]], 0)
    return np.ascontiguousarray(qb), np.ascontiguousarray(cb)


def prep_A(points_b, weights, half, R):
    """Inputs for one phase-A core: query rows [half*R, (half+1)*R)."""
    N = points_b.shape[1]
    w1 = np.asarray(weights["w2d_0"])
    b1 = np.asarray(weights["b2d_0"])
    w1a, w1b = w1[:, 0:3], w1[:, 3:6]
    w1cT = np.concatenate([(w1a - w1b).T, b1[None, :]], 0).astype(np.float32)
    qb, cb = _qb_cb(points_b)
    sl = slice(half * R, (half + 1) * R)
    pts = points_b.astype(np.float32)
    ptsq = np.concatenate([pts[:, sl], np.ones((1, R), np.float32)], 0)
    tbl = np.concatenate([pts.T, np.zeros((N, 1), np.float32)], 1)
    return {
        "pts": np.ascontiguousarray(ptsq),
        "qb": np.ascontiguousarray(qb[:, sl]),
        "cb": cb,
        "tbl": np.ascontiguousarray(tbl),
        "w1bTg": np.ascontiguousarray(_w1big(w1b, w1cT)),
    }


def prep_B(x1s, s1, t1, weights):
    w2T = np.asarray(weights["w2d_1"]).T.astype(np.float32)   # (32, 64)
    w2bd = np.zeros((128, 128), np.float32)
    for rep in range(2):
        w2bd[64 * rep + 0:64 * rep + 32, 0:64] = w2T
        w2bd[64 * rep + 32:64 * rep + 64, 64:128] = w2T
    return {
        "x1s": x1s,
        "s1t": np.ascontiguousarray(np.tile(s1.reshape(-1), 4).reshape(-1, 1).astype(np.float32)),
        "t1t": np.ascontiguousarray(np.tile(t1.reshape(-1), 4).reshape(-1, 1).astype(np.float32)),
        "w2bd": np.ascontiguousarray(w2bd),
    }


def host_gn1(accx1, accx1sq, g1, t1g, NK):
    """Host-side GN1 scale/shift from merged per-channel partial sums."""
    mean1 = accx1.sum(1, dtype=np.float64) / NK
    ex2 = accx1sq.sum(1, dtype=np.float64) / NK
    gm = mean1.reshape(8, 4).mean(1)
    gex2 = ex2.reshape(8, 4).mean(1)
    var = gex2 - gm * gm
    s = np.asarray(g1, np.float64) / np.sqrt(var.repeat(4) + EPS)
    t = np.asarray(t1g, np.float64) - s * gm.repeat(4)
    return s.astype(np.float32), t.astype(np.float32)


def host_gn2(acczs, accysqs, weights, NK):
    """GN2 scale/shift from the two cores' partial sums of one batch."""
    w2 = np.asarray(weights["w2d_1"], np.float64)            # (64, 32)
    b2f = (np.asarray(weights["b2d_1"], np.float64)
           - w2.sum(1))                                       # +1 fold
    zsum = np.zeros(32, np.float64)
    eysq = np.zeros(64, np.float64)
    for accz, accysq in zip(acczs, accysqs):
        zsum += accz.astype(np.float64).reshape(4, 32, -1).sum((0, 2))
        eysq += accysq.astype(np.float64).reshape(2, 64, -1).sum((0, 2))
    ymean = (w2 @ zsum) / NK
    mean_t = ymean + b2f
    e2_t = eysq / NK + 2.0 * b2f * ymean + b2f * b2f
    gm = mean_t.reshape(8, 8).mean(1)
    ge2 = e2_t.reshape(8, 8).mean(1)
    var = ge2 - gm * gm
    g2 = np.asarray(weights["g2d_1"], np.float64)
    t2 = np.asarray(weights["t2d_1"], np.float64)
    s = g2 / np.sqrt(var.repeat(8) + EPS)
    # out = g*((y+b2f) - gm)/sd + t2 = s*y + (t2 - s*gm + s*b2f)
    t = t2 - s * gm.repeat(8) + s * b2f
    return s.astype(np.float32), t.astype(np.float32)


def prep_C(ypool, s2, t2, weights, hh):
    w3T = np.asarray(weights["w1d_0"]).T.astype(np.float32)   # (64, 128)
    w3T2 = np.concatenate([w3T, w3T], 0)                      # (128, 128)
    w4T = np.asarray(weights["w1d_1"]).T.astype(np.float32)   # (128, 256)
    b3f = (np.asarray(weights["b1d_0"])
           - np.asarray(weights["w1d_0"]).sum(1))
    b4f = (np.asarray(weights["b1d_1"])
           - np.asarray(weights["w1d_1"]).sum(1))
    sl4 = slice(hh * 128, (hh + 1) * 128)
    m = {
        "ypool": np.ascontiguousarray(ypool),
        "s2t": np.ascontiguousarray(np.tile(s2, 2).reshape(-1, 1).astype(np.float32)),
        "t2t": np.ascontiguousarray(np.tile(t2, 2).reshape(-1, 1).astype(np.float32)),
        "w3T2": np.ascontiguousarray(w3T2),
        "w4h": np.ascontiguousarray(w4T[:, sl4]),
        "b3": b3f.reshape(-1, 1).astype(np.float32),
        "g3": np.asarray(weights["g1d_0"]).reshape(-1, 1).astype(np.float32),
        "t3": np.asarray(weights["t1d_0"]).reshape(-1, 1).astype(np.float32),
        "b4": np.ascontiguousarray(b4f[sl4].reshape(-1, 1).astype(np.float32)),
        "g4": np.ascontiguousarray(np.asarray(weights["g1d_1"])[sl4].reshape(-1, 1).astype(np.float32)),
        "t4": np.ascontiguousarray(np.asarray(weights["t1d_1"])[sl4].reshape(-1, 1).astype(np.float32)),
    }
    for nm, (cc, gg) in (("c128", (128, 8)), ("c128h", (128, 4))):
        indT, bindT = _inds(cc, gg)
        m[f"indT_{nm}"] = indT
        m[f"bindT_{nm}"] = bindT
    return m


_CACHE = {}


def _build_A(N, R, ch=512):
    key = ("A", N, R, ch)
    if key in _CACHE:
        return _CACHE[key]
    nc = bacc.Bacc("TRN2", target_bir_lowering=False, debug=False,
                   num_devices=8)
    NTh = R // 128
    ins = {
        "pts": nc.dram_tensor("pts", [4, R], F32, kind="ExternalInput").ap(),
        "qb": nc.dram_tensor("qb", [21, R], BF, kind="ExternalInput").ap(),
        "cb": nc.dram_tensor("cb", [21, N], BF, kind="ExternalInput").ap(),
        "tbl": nc.dram_tensor("tbl", [N, 4], F32, kind="ExternalInput").ap(),
        "w1bTg": nc.dram_tensor("w1bTg", [68, 32 * 16], F32,
                                kind="ExternalInput").ap(),
    }
    outs = {
        "x1o": nc.dram_tensor("x1o", [NTh, 32, 128 * K], F32,
                              kind="ExternalOutput").ap(),
        "accx1o": nc.dram_tensor("accx1o", [32, NTh * 4], F32,
                                 kind="ExternalOutput").ap(),
        "accx1sqo": nc.dram_tensor("accx1sqo", [32, NTh * 4], F32,
                                   kind="ExternalOutput").ap(),
    }
    with tile.TileContext(nc) as tc:
        build_phaseA(tc, outs, ins, N, R, ch)
    nc.compile()
    _CACHE[key] = nc
    return nc


def _build_B(R):
    key = ("B", R)
    if key in _CACHE:
        return _CACHE[key]
    nc = bacc.Bacc("TRN2", target_bir_lowering=False, debug=False,
                   num_devices=8)
    NTh = R // 128
    NP = R // 512
    ins = {
        "x1s": nc.dram_tensor("x1s", [NTh, 32, 128 * K], F32,
                              kind="ExternalInput").ap(),
        "s1t": nc.dram_tensor("s1t", [128, 1], F32, kind="ExternalInput").ap(),
        "t1t": nc.dram_tensor("t1t", [128, 1], F32, kind="ExternalInput").ap(),
        "w2bd": nc.dram_tensor("w2bd", [128, 128], F32,
                               kind="ExternalInput").ap(),
    }
    outs = {
        "ypoolo": nc.dram_tensor("ypoolo", [64, R], F32,
                                 kind="ExternalOutput").ap(),
        "acczo": nc.dram_tensor("acczo", [128, 2 * NP], F32,
                                kind="ExternalOutput").ap(),
        "accysqo": nc.dram_tensor("accysqo", [128, 2 * NP], F32,
                                  kind="ExternalOutput").ap(),
    }
    with tile.TileContext(nc) as tc:
        build_phaseB(tc, outs, ins, R)
    nc.compile()
    _CACHE[key] = nc
    return nc


def _build_C(N):
    key = ("C", N)
    if key in _CACHE:
        return _CACHE[key]
    nc = bacc.Bacc("TRN2", target_bir_lowering=False, debug=False,
                   num_devices=8)
    ins = {"ypool": nc.dram_tensor("ypool", [64, N], F32,
                                   kind="ExternalInput").ap()}
    specs = [("s2t", [128, 1]), ("t2t", [128, 1]), ("w3T2", [128, 128]),
             ("w4h", [128, 128]), ("b3", [128, 1]), ("g3", [128, 1]),
             ("t3", [128, 1]), ("b4", [128, 1]), ("g4", [128, 1]),
             ("t4", [128, 1]),
             ("indT_c128", [128, 8]), ("bindT_c128", [8, 128]),
             ("indT_c128h", [128, 4]), ("bindT_c128h", [4, 128])]
    for nm, shape in specs:
        ins[nm] = nc.dram_tensor(nm, shape, F32, kind="ExternalInput").ap()
    outs = {"out": nc.dram_tensor("out", [128, N], F32,
                                  kind="ExternalOutput").ap()}
    with tile.TileContext(nc) as tc:
        build_phaseC(tc, outs, ins, N)
    nc.compile()
    _CACHE[key] = nc
    return nc


def kernel(points, _trace=False, **weights):
    points = np.asarray(points)
    Bn, _, N = points.shape
    R = N // 2
    ncA = _build_A(N, R)
    ncB = _build_B(R)
    ncC = _build_C(N)

    if _trace:
        import tempfile

    def _run(ncX, in_maps, prefix):
        kw = {}
        if _trace:
            kw = dict(trace=True, tmpdir=tempfile.mkdtemp(prefix=prefix))
        return run_bass_kernel_spmd(ncX, in_maps, core_ids=list(range(8)),
                                    **kw)

    in_maps_A = []
    for c in range(8):
        in_maps_A.append(prep_A(points[(c // 2) % Bn], weights, c % 2, R))
    resA = _run(ncA, in_maps_A, "trA_")

    g1 = np.asarray(weights["g2d_0"])
    t1g = np.asarray(weights["t2d_0"])
    in_maps_B = []
    for b in range(Bn):
        e, o = resA.results[2 * b], resA.results[2 * b + 1]
        acc = np.concatenate([e["accx1o"], o["accx1o"]], 1)
        accsq = np.concatenate([e["accx1sqo"], o["accx1sqo"]], 1)
        s1, t1 = host_gn1(acc, accsq, g1, t1g, N * K)
        for half in range(2):
            x1s = np.ascontiguousarray(resA.results[2 * b + half]["x1o"])
            in_maps_B.append(prep_B(x1s, s1, t1, weights))
    resB = _run(ncB, in_maps_B, "trB_")

    in_maps_C = []
    for b in range(Bn):
        e, o = resB.results[2 * b], resB.results[2 * b + 1]
        s2, t2 = host_gn2([e["acczo"], o["acczo"]],
                          [e["accysqo"], o["accysqo"]], weights, N * K)
        ypool = np.concatenate([e["ypoolo"], o["ypoolo"]], 1)
        for hh in range(2):
            in_maps_C.append(prep_C(ypool, s2, t2, weights, hh))
    resC = _run(ncC, in_maps_C, "trC_")

    out = np.stack([
        np.concatenate([resC.results[2 * b]["out"],
                        resC.results[2 * b + 1]["out"]], 0)
        for b in range(Bn)], 0)
    if _trace:
        kernel.last_exec_A = resA.exec_time_ns
        kernel.last_exec_B = resB.exec_time_ns
        kernel.last_exec_C = resC.exec_time_ns
        kernel.last_exec_ns = ((resA.exec_time_ns or 0)
                               + (resB.exec_time_ns or 0)
                               + (resC.exec_time_ns or 0))
        kernel.trace_A = resA.instructions_and_trace
        kernel.trace_B = resB.instructions_and_trace
        kernel.trace_C = resC.instructions_and_trace
    return out.astype(np.float32)


# revision 13
# speedup vs baseline: 1.6447x; 1.0428x over previous
"""Trainium2 Bass kernel for nn_AbsoluteRelativePositionEmbedding.

Three SPMD launches over 8 NeuronCores:
  Launch A (8 cores): each core handles HALF of one batch's query rows.
    kNN(16) over all N=8192 candidates -> gather neighbor coords ->
    conv1(6->32) -> x1 + per-channel partial sums (for GroupNorm-1 stats).
  Host: merges the tiny GN1 partial sums per batch -> scale/shift vectors.
  Launch B (8 cores, row-split): each core processes its OWN phase-A rows:
    gn1+ELU -> conv2(32->64, block-diag float32r) -> max over K -> ypool
    half + GN2 partial sums. Elementwise work is packed 4 row-tiles deep so
    all 128 partitions are busy.
  Host: merges GN2 partials -> s2/t2.
  Launch C (8 cores): full batch per core, channel-split final conv:
    gn2+ELU -> conv3(64->128) -> gn3 (on device) + ELU -> conv4 half
    (128 of 256 out channels) -> gn4 (on device) + ELU -> out half.

The distance matmul uses a 3-level bf16 split (21 contraction rows) which
matches fp32 accuracy at 4x the PE rate. ELU(x)+1 = min(exp(x),1)+relu(x)
is used internally; the +1 is folded into the next conv's bias.
"""
import sys
sys.path.insert(0, '/opt/trn_rl_repo')

import numpy as np
import ml_dtypes

import concourse.bass as bass
import concourse.bacc as bacc
import concourse.mybir as mybir
import concourse.tile as tile
from concourse.bass import IndirectOffsetOnAxis
from concourse.bass_utils import run_bass_kernel_spmd
from concourse.masks import make_identity

F32 = mybir.dt.float32
F32R = mybir.dt.float32r
U32 = mybir.dt.uint32
BF = mybir.dt.bfloat16
AF = mybir.ActivationFunctionType
ALU = mybir.AluOpType
AX = mybir.AxisListType

K = 16
EPS = 1e-5
NEG = -1e30
GROUPS = 8


def _r(ap):
    """float32r view for full-rate fp32 matmuls."""
    return ap.bitcast(F32R)


def _elu1_affine_inplace(nc, pool, x, s, t):
    """x = elu(s*x + t) + 1 in place, using shared [128, f] scratch tags."""
    p, f = x.shape[0], x.shape[1]
    m = pool.tile([128, f], F32, tag="elu_scr_m")
    e = pool.tile([128, f], F32, tag="elu_scr_e")
    nc.scalar.activation(m[0:p, :], x, AF.Relu, scale=s, bias=t)
    nc.scalar.activation(e[0:p, :], x, AF.Exp, scale=s, bias=t)
    nc.vector.scalar_tensor_tensor(x, e[0:p, :], 1.0, m[0:p, :], ALU.min,
                                   ALU.add)


def _gn_coeffs(tc, nc, stats_pool, C, G, mean_c, ex2_c, bvec, gvec, tvec,
               indT, bindT, name, eps_ap):
    """Per-channel GN scale s / shift t so that gn(y+b) = s*y + t."""
    psp = tc.alloc_tile_pool(name=f"ps_{name}", bufs=1, space="PSUM")
    m1 = stats_pool.tile([C, 1], F32, tag=f"m1_{name}")
    m2 = stats_pool.tile([C, 1], F32, tag=f"m2_{name}")
    nc.vector.tensor_add(m1, mean_c, bvec)
    tmp = stats_pool.tile([C, 1], F32, tag=f"tmp_{name}")
    nc.vector.tensor_add(tmp, mean_c, m1)
    nc.vector.tensor_mul(tmp, tmp, bvec)
    nc.vector.tensor_add(m2, ex2_c, tmp)
    gm_ps = psp.tile([G, 1], F32, tag="gm_ps")
    gm_sb = stats_pool.tile([G, 1], F32, tag=f"gm_sb_{name}")
    bc1 = stats_pool.tile([C, 1], F32, tag=f"bc1_{name}")
    bc2 = stats_pool.tile([C, 1], F32, tag=f"bc2_{name}")
    bc_ps = psp.tile([C, 1], F32, tag="bc_ps")
    nc.tensor.matmul(gm_ps, indT, m1, start=True, stop=True)
    nc.scalar.copy(gm_sb, gm_ps)
    nc.tensor.matmul(bc_ps, bindT, gm_sb, start=True, stop=True)
    nc.scalar.copy(bc1, bc_ps)
    nc.tensor.matmul(gm_ps, indT, m2, start=True, stop=True)
    nc.scalar.copy(gm_sb, gm_ps)
    nc.tensor.matmul(bc_ps, bindT, gm_sb, start=True, stop=True)
    nc.scalar.copy(bc2, bc_ps)
    var = stats_pool.tile([C, 1], F32, tag=f"var_{name}")
    nc.vector.tensor_mul(tmp, bc1, bc1)
    nc.vector.tensor_sub(var, bc2, tmp)
    sd = stats_pool.tile([C, 1], F32, tag=f"sd_{name}")
    nc.scalar.activation(sd, var, AF.Sqrt, bias=eps_ap[0:C, :])
    s = stats_pool.tile([C, 1], F32, tag=f"s_{name}")
    nc.vector.reciprocal(s, sd)
    nc.vector.tensor_mul(s, s, gvec)
    t = stats_pool.tile([C, 1], F32, tag=f"t_{name}")
    nc.vector.tensor_sub(tmp, bc1, bvec)
    nc.vector.tensor_mul(tmp, s, tmp)
    nc.vector.tensor_sub(t, tvec, tmp)
    psp.release()
    return s, t


def build_phaseA(tc, outs, ins, N, R, ch=1024):
    """kNN + gather + conv1 for R query rows against N candidates."""
    nc = tc.nc
    NTh = R // 128
    NCH = N // ch

    tbl_d = ins["tbl"]
    x1o_d = outs["x1o"]

    consts = tc.alloc_tile_pool(name="consts", bufs=1)
    persist = tc.alloc_tile_pool(name="persist", bufs=1)

    w1bTg = consts.tile([68, 32 * 16], F32, tag="w1bTg")
    nc.sync.dma_start(w1bTg, ins["w1bTg"])
    ident = consts.tile([128, 128], F32, tag="ident")
    make_identity(nc, ident)

    Q = persist.tile([4, R], F32, tag="Q")
    Qb = persist.tile([21, R], BF, tag="Qb")
    Cb = persist.tile([21, N], BF, tag="Cb")
    nc.sync.dma_start(Q, ins["pts"])
    nc.sync.dma_start(Qb, ins["qb"])
    nc.sync.dma_start(Cb, ins["cb"])

    accx1 = persist.tile([32, NTh * 4], F32, tag="accx1")
    accx1sq = persist.tile([32, NTh * 4], F32, tag="accx1sq")

    with tc.tile_pool(name="pa_sb", bufs=2) as pa, \
         tc.tile_pool(name="pa_nd", bufs=2) as pa_nd, \
         tc.tile_pool(name="pa_ps", bufs=2, space="PSUM") as psa, \
         tc.tile_pool(name="pa_ps1", bufs=2, space="PSUM") as psa1, \
         tc.tile_pool(name="pa_pst", bufs=2, space="PSUM") as psat:
        for t in range(NTh):
            r0 = t * 128
            ndsb = pa_nd.tile([128, N], F32, tag="ndsb")
            CW = min(1024, N)
            for cc in range(N // CW):
                nd_ps = psa.tile([128, CW], F32, tag="nd_ps")
                for h in range(CW // 512):
                    nc.tensor.matmul(
                        nd_ps[:, h * 512:(h + 1) * 512],
                        Qb[:, r0:r0 + 128],
                        Cb[:, cc * CW + h * 512:cc * CW + (h + 1) * 512],
                        start=True, stop=True)
                nc.scalar.copy(ndsb[:, cc * CW:(cc + 1) * CW], nd_ps)
            # top-8 per chunk
            cand = pa.tile([128, NCH * 8], F32, tag="cand")
            for cq in range(NCH):
                nc.vector.max(out=cand[:, cq * 8:(cq + 1) * 8],
                              in_=ndsb[:, cq * ch:(cq + 1) * ch])
            r1 = pa.tile([128, 8], F32, tag="r1")
            r2 = pa.tile([128, 8], F32, tag="r2")
            r3 = pa.tile([128, 8], F32, tag="r3")
            cand2 = pa.tile([128, NCH * 8], F32, tag="cand2")
            cand3 = pa.tile([128, NCH * 8], F32, tag="cand3")
            nc.vector.max(out=r1, in_=cand)
            nc.vector.match_replace(out=cand2, in_to_replace=r1,
                                    in_values=cand, imm_value=NEG)
            nc.vector.max(out=r2, in_=cand2)
            nc.vector.match_replace(out=cand3, in_to_replace=r2,
                                    in_values=cand2, imm_value=NEG)
            nc.vector.max(out=r3, in_=cand3)
            # winners: ranks 2..17 (rank 1 is self)
            wA = pa.tile([128, 8], F32, tag="wA")
            nc.vector.tensor_copy(wA[:, 0:7], r1[:, 1:8])
            nc.vector.tensor_copy(wA[:, 7:8], r3[:, 0:1])
            gidx = pa.tile([128, K], U32, tag="gidx")
            nc.vector.max_index(gidx[:, 0:8], wA, ndsb)
            nc.vector.max_index(gidx[:, 8:16], r2, ndsb)
            # gather neighbor coords (16B rows from the xyz0 table)
            gt = pa.tile([128, K * 4], F32, tag="gt")
            for k in range(K):
                nc.gpsimd.indirect_dma_start(
                    out=gt[:, 4 * k:4 * (k + 1)], out_offset=None,
                    in_=tbl_d,
                    in_offset=IndirectOffsetOnAxis(ap=gidx[:, k:k + 1],
                                                   axis=0))
            gtT_ps = psat.tile([64, 128], F32, tag="gtT_ps")
            nc.tensor.transpose(gtT_ps, gt, ident)
            rhs68 = pa.tile([68, 128], F32, tag="rhs68")
            nc.scalar.copy(rhs68[0:64, :], gtT_ps)
            nc.scalar.copy(rhs68[64:68, :], Q[:, r0:r0 + 128])
            # conv1 into (32, 128*K), position order (k, r)
            x1sb = pa.tile([32, 128 * K], F32, tag="x1sb")
            for q in range(K // 4):
                x1_ps = psa1.tile([32, 512], F32, tag="x1_ps")
                for j in range(4):
                    k = 4 * q + j
                    nc.tensor.matmul(x1_ps[:, j * 128:(j + 1) * 128],
                                     w1bTg[:, 32 * k:32 * (k + 1)], rhs68,
                                     start=True, stop=True)
                sl = slice(q * 512, (q + 1) * 512)
                nc.scalar.activation(
                    x1sb[:, sl], x1_ps, AF.Identity,
                    accum_out=accx1[:, 4 * t + q:4 * t + q + 1])
                sqsc = pa.tile([32, 512], F32, tag="sqsc")
                nc.scalar.activation(
                    sqsc, x1sb[:, sl], AF.Square,
                    accum_out=accx1sq[:, 4 * t + q:4 * t + q + 1])
            nc.sync.dma_start(x1o_d[t], x1sb)

    nc.sync.dma_start(outs["accx1o"], accx1)
    nc.sync.dma_start(outs["accx1sqo"], accx1sq)
    persist.release()
    consts.release()


def build_phaseB(tc, outs, ins, R):
    """Row-split: gn1+ELU -> conv2 (block-diag) -> max over K -> ypool half.

    x1 tiles are packed 4-deep on partitions: partition 32j+c holds channel
    c of row-tile 4t+j. conv2 runs as two block-diagonal (64->128) matmuls.
    Emits per-core GN2 partial sums (z sums, y^2 sums) for the host merge.
    """
    nc = tc.nc
    NP = R // 512                  # packed tiles (4 row-tiles each)

    x1s_d = ins["x1s"]
    yp_d = outs["ypoolo"]

    consts = tc.alloc_tile_pool(name="consts", bufs=1)
    persist = tc.alloc_tile_pool(name="persist", bufs=1)

    s1t = consts.tile([128, 1], F32, tag="s1t")
    t1t = consts.tile([128, 1], F32, tag="t1t")
    w2bdf = consts.tile([128, 128], F32, tag="w2bdf")
    w2bd = consts.tile([128, 128], F32R, tag="w2bd")
    nc.sync.dma_start(s1t, ins["s1t"])
    nc.sync.dma_start(t1t, ins["t1t"])
    nc.sync.dma_start(w2bdf, ins["w2bd"])
    nc.scalar.copy(w2bd, w2bdf)

    accz = persist.tile([128, 2 * NP], F32, tag="accz")
    accysq = persist.tile([128, 2 * NP], F32, tag="accysq")

    with tc.tile_pool(name="pb_sb", bufs=2) as pb, \
         tc.tile_pool(name="pb_scr", bufs=1) as pbs, \
         tc.tile_pool(name="pb_ps", bufs=2, space="PSUM") as psb:
        for t in range(NP):
            xp = pb.tile([128, 2048], F32, tag="xp")
            nc.sync.dma_start(xp, x1s_d[t])
            m_ = pb.tile([128, 2048], F32, tag="elu_m")
            e_ = pb.tile([128, 2048], F32, tag="elu_e")
            z = pb.tile([128, 2048], F32R, tag="z")
            # z = elu(gn1(x1)) + 1  (the +1 is folded into b2' on host)
            for q in range(2):
                sl = slice(q * 1024, (q + 1) * 1024)
                nc.scalar.activation(m_[:, sl], xp[:, sl], AF.Relu,
                                     scale=s1t, bias=t1t)
                nc.scalar.activation(e_[:, sl], xp[:, sl], AF.Exp,
                                     scale=s1t, bias=t1t)
                nc.vector.scalar_tensor_tensor(
                    z[:, sl], e_[:, sl], 1.0, m_[:, sl], ALU.min, ALU.add,
                    accum_out=accz[:, 2 * t + q:2 * t + q + 1])
            sqscr = pb.tile([128, 2048], F32, tag="sqscr")
            for h in range(2):     # row-tiles (4t+2h, 4t+2h+1)
                y_ps = psb.tile([128, 2048], F32, tag="y_ps")
                for q in range(4):
                    sl = slice(q * 512, (q + 1) * 512)
                    nc.tensor.matmul(y_ps[:, sl],
                                     w2bd[64 * h:64 * (h + 1), :],
                                     z[64 * h:64 * (h + 1), sl],
                                     start=True, stop=True)
                nc.scalar.activation(sqscr, y_ps, AF.Square,
                                     accum_out=accysq[:, 2 * t + h:2 * t + h + 1])
                # max over K (position order (k, r): fold k halves)
                p1 = pb.tile([128, 1024], F32, tag="p1")
                p2 = pb.tile([128, 512], F32, tag="p2")
                p3 = pb.tile([128, 256], F32, tag="p3")
                p4 = pb.tile([128, 128], F32, tag="p4")
                yh = pb.tile([128, 1024], F32, tag="yh")
                nc.scalar.copy(yh, y_ps[:, 1024:2048])
                nc.vector.tensor_tensor(p1, y_ps[:, 0:1024],
                                        yh, op=ALU.max)
                nc.vector.tensor_tensor(p2, p1[:, 0:512], p1[:, 512:1024],
                                        op=ALU.max)
                nc.vector.tensor_tensor(p3, p2[:, 0:256], p2[:, 256:512],
                                        op=ALU.max)
                nc.vector.tensor_tensor(p4, p3[:, 0:128], p3[:, 128:256],
                                        op=ALU.max)
                m = 2 * t + h
                nc.sync.dma_start(yp_d[:, m * 128:(m + 1) * 128], p4)

    nc.sync.dma_start(outs["acczo"], accz)
    nc.sync.dma_start(outs["accysqo"], accysq)
    persist.release()
    consts.release()


def _elu1_chunked(nc, pool, x, s, t, xin=None, nch=4):
    """x_out = elu(s*xin + t) + 1, chunked so scalar/vector pipeline."""
    p, f = x.shape[0], x.shape[1]
    if xin is None:
        xin = x.bitcast(F32) if x.dtype != F32 else x
    cw = f // nch
    m = pool.tile([128, f], F32, tag="elu_scr_m")
    e = pool.tile([128, f], F32, tag="elu_scr_e")
    for c in range(nch):
        sl = slice(c * cw, (c + 1) * cw)
        nc.scalar.activation(m[0:p, sl], xin[:, sl], AF.Relu, scale=s,
                             bias=t)
        nc.scalar.activation(e[0:p, sl], xin[:, sl], AF.Exp, scale=s,
                             bias=t)
        nc.vector.scalar_tensor_tensor(x[:, sl], e[0:p, sl], 1.0,
                                       m[0:p, sl], ALU.min, ALU.add)


def build_phaseC(tc, outs, ins, N):
    """Full batch: gn2+ELU -> conv3 -> gn3+ELU -> conv4 half -> gn4+ELU.

    ypool is packed 2-deep on partitions (partition 64h+c = channel c for
    column half h). conv4 computes only this core's 128 of 256 channels;
    its 4 GN groups are self-contained so gn4 runs fully on device.
    """
    nc = tc.nc
    NH = N // 2

    out_d = outs["out"]

    consts = tc.alloc_tile_pool(name="consts", bufs=1)
    stats_pool = tc.alloc_tile_pool(name="stats", bufs=1)
    persist = tc.alloc_tile_pool(name="persist", bufs=1)

    def load_const(name, shape):
        t = consts.tile(shape, F32, tag=name)
        nc.sync.dma_start(t, ins[name])
        return t

    s2t = load_const("s2t", [128, 1])
    t2t = load_const("t2t", [128, 1])
    w3T2f = load_const("w3T2", [128, 128])
    w4hf = load_const("w4h", [128, 128])
    w3T2 = consts.tile([128, 128], F32R, tag="w3T2r")
    w4h = consts.tile([128, 128], F32R, tag="w4hr")
    nc.scalar.copy(w3T2, w3T2f)
    nc.scalar.copy(w4h, w4hf)
    b3 = load_const("b3", [128, 1])
    g3 = load_const("g3", [128, 1])
    t3 = load_const("t3", [128, 1])
    b4 = load_const("b4", [128, 1])
    g4 = load_const("g4", [128, 1])
    t4 = load_const("t4", [128, 1])
    ind3T = load_const("indT_c128", [128, 8])
    bind3T = load_const("bindT_c128", [8, 128])
    ind4T = load_const("indT_c128h", [128, 4])
    bind4T = load_const("bindT_c128h", [4, 128])
    eps_t = consts.tile([128, 1], F32, tag="eps_t")
    nc.vector.memset(eps_t, EPS)

    with tc.tile_pool(name="pc_sb", bufs=1) as pc, \
         tc.tile_pool(name="pc_ps", bufs=2, space="PSUM") as psc:
        yp = pc.tile([128, NH], F32, tag="yp")
        nc.sync.dma_start(yp[0:64, :], ins["ypool"][:, 0:NH])
        nc.sync.dma_start(yp[64:128, :], ins["ypool"][:, NH:N])
        u = pc.tile([128, NH], F32R, tag="u")
        _elu1_chunked(nc, pc, u, s2t, t2t, xin=yp)

        NCH = NH // 1024
        vs = []
        accv = stats_pool.tile([128, 2 * NCH], F32, tag="accv")
        accvsq = stats_pool.tile([128, 2 * NCH], F32, tag="accvsq")
        sqscr = pc.tile([128, 1024], F32, tag="sqscr")
        for hh in range(2):
            v = pc.tile([128, NH], F32R, tag=f"v{hh}")
            for cch in range(NCH):
                sl = slice(cch * 1024, (cch + 1) * 1024)
                v_ps = psc.tile([128, 1024], F32, tag="mm_ps")
                for q in range(2):
                    sq_ = slice(cch * 1024 + q * 512,
                                cch * 1024 + (q + 1) * 512)
                    nc.tensor.matmul(v_ps[:, q * 512:(q + 1) * 512],
                                     w3T2[64 * hh:64 * (hh + 1), :],
                                     u[64 * hh:64 * (hh + 1), sq_],
                                     start=True, stop=True)
                ci = hh * NCH + cch
                nc.scalar.activation(v[:, sl], v_ps, AF.Identity,
                                     accum_out=accv[:, ci:ci + 1])
                nc.scalar.activation(sqscr, v.bitcast(F32)[:, sl], AF.Square,
                                     accum_out=accvsq[:, ci:ci + 1])
            vs.append(v)
        mean3 = stats_pool.tile([128, 1], F32, tag="mean3")
        ex23 = stats_pool.tile([128, 1], F32, tag="ex23")
        nc.vector.tensor_reduce(mean3, accv, AX.X, ALU.add)
        nc.vector.tensor_reduce(ex23, accvsq, AX.X, ALU.add)
        nc.vector.tensor_scalar_mul(mean3, mean3, 1.0 / N)
        nc.vector.tensor_scalar_mul(ex23, ex23, 1.0 / N)
        s3, t3c = _gn_coeffs(tc, nc, stats_pool, 128, GROUPS, mean3, ex23,
                             b3, g3, t3, ind3T, bind3T, name="gn3",
                             eps_ap=eps_t)
        for v in vs:
            _elu1_chunked(nc, pc, v, s3, t3c)

        os_ = []
        acco = stats_pool.tile([128, 2 * NCH], F32, tag="acco")
        accosq = stats_pool.tile([128, 2 * NCH], F32, tag="accosq")
        for hh in range(2):
            o = pc.tile([128, NH], F32, tag=f"o{hh}")
            for cch in range(NCH):
                sl = slice(cch * 1024, (cch + 1) * 1024)
                o_ps = psc.tile([128, 1024], F32, tag="mm_ps")
                for q in range(2):
                    sq_ = slice(cch * 1024 + q * 512,
                                cch * 1024 + (q + 1) * 512)
                    nc.tensor.matmul(o_ps[:, q * 512:(q + 1) * 512],
                                     w4h, vs[hh][:, sq_],
                                     start=True, stop=True)
                ci = hh * NCH + cch
                nc.scalar.activation(o[:, sl], o_ps, AF.Identity,
                                     accum_out=acco[:, ci:ci + 1])
                nc.scalar.activation(sqscr, o[:, sl], AF.Square,
                                     accum_out=accosq[:, ci:ci + 1])
            os_.append(o)
        mean4 = stats_pool.tile([128, 1], F32, tag="mean4")
        ex24 = stats_pool.tile([128, 1], F32, tag="ex24")
        nc.vector.tensor_reduce(mean4, acco, AX.X, ALU.add)
        nc.vector.tensor_reduce(ex24, accosq, AX.X, ALU.add)
        nc.vector.tensor_scalar_mul(mean4, mean4, 1.0 / N)
        nc.vector.tensor_scalar_mul(ex24, ex24, 1.0 / N)
        s4, t4c = _gn_coeffs(tc, nc, stats_pool, 128, 4, mean4, ex24,
                             b4, g4, t4, ind4T, bind4T, name="gn4",
                             eps_ap=eps_t)
        for hh in range(2):
            o = os_[hh]
            _elu1_chunked(nc, pc, o, s4, t4c)
            nc.vector.tensor_scalar_add(o, o, -1.0)
            nc.sync.dma_start(out_d[:, hh * NH:(hh + 1) * NH], o)

    persist.release()
    stats_pool.release()
    consts.release()


# ---------------- host-side prep ----------------

def _inds(Cc, Gg):
    gs = Cc // Gg
    indT = np.zeros((Cc, Gg), np.float32)
    bindT = np.zeros((Gg, Cc), np.float32)
    for g in range(Gg):
        indT[g * gs:(g + 1) * gs, g] = 1.0 / gs
        bindT[g, g * gs:(g + 1) * gs] = 1.0
    return indT, bindT


def _w1big(w1b, w1cT):
    big = np.zeros((68, 32 * 16), np.float32)
    for k in range(16):
        big[4 * k:4 * k + 3, 32 * k:32 * (k + 1)] = w1b.T
        big[64:68, 32 * k:32 * (k + 1)] = w1cT
    return big


def _split3(x):
    h = x.astype(ml_dtypes.bfloat16)
    r = x.astype(np.float32) - h.astype(np.float32)
    m = r.astype(ml_dtypes.bfloat16)
    l = (r - m.astype(np.float32)).astype(ml_dtypes.bfloat16)
    return h, m, l


def _qb_cb(points_b):
    p = points_b.astype(np.float64)
    sq = (p * p).sum(0)
    h, m, l = _split3(points_b.astype(np.float32))
    sh, sm, sl = _split3(sq.astype(np.float32))
    BFD = ml_dtypes.bfloat16
    ones = np.ones_like(h[0:1])
    qb = np.concatenate([h, h, m, m, h, l, ones, ones, ones], 0)
    cb = np.concatenate([(2 * h.astype(np.float32)).astype(BFD),
                         (2 * m.astype(np.float32)).astype(BFD),
                         (2 * h.astype(np.float32)).astype(BFD),
                         (2 * m.astype(np.float32)).astype(BFD),
                         (2 * l.astype(np.float32)).astype(BFD),
                         (2 * h.astype(np.float32)).astype(BFD),
                         -sh[None, :], -sm[None, :], -sl[None, :]], 0)
    return np.ascontiguousarray(qb), np.ascontiguousarray(cb)


def prep_A(points_b, weights, half, R):
    """Inputs for one phase-A core: query rows [half*R, (half+1)*R)."""
    N = points_b.shape[1]
    w1 = np.asarray(weights["w2d_0"])
    b1 = np.asarray(weights["b2d_0"])
    w1a, w1b = w1[:, 0:3], w1[:, 3:6]
    w1cT = np.concatenate([(w1a - w1b).T, b1[None, :]], 0).astype(np.float32)
    qb, cb = _qb_cb(points_b)
    sl = slice(half * R, (half + 1) * R)
    pts = points_b.astype(np.float32)
    ptsq = np.concatenate([pts[:, sl], np.ones((1, R), np.float32)], 0)
    tbl = np.concatenate([pts.T, np.zeros((N, 1), np.float32)], 1)
    return {
        "pts": np.ascontiguousarray(ptsq),
        "qb": np.ascontiguousarray(qb[:, sl]),
        "cb": cb,
        "tbl": np.ascontiguousarray(tbl),
        "w1bTg": np.ascontiguousarray(_w1big(w1b, w1cT)),
    }


def prep_B(x1s, s1, t1, weights):
    w2T = np.asarray(weights["w2d_1"]).T.astype(np.float32)   # (32, 64)
    w2bd = np.zeros((128, 128), np.float32)
    for rep in range(2):
        w2bd[64 * rep + 0:64 * rep + 32, 0:64] = w2T
        w2bd[64 * rep + 32:64 * rep + 64, 64:128] = w2T
    NP = x1s.shape[0] // 4
    return {
        "x1s": np.ascontiguousarray(x1s.reshape(NP, 128, -1)),
        "s1t": np.ascontiguousarray(np.tile(s1.reshape(-1), 4).reshape(-1, 1).astype(np.float32)),
        "t1t": np.ascontiguousarray(np.tile(t1.reshape(-1), 4).reshape(-1, 1).astype(np.float32)),
        "w2bd": np.ascontiguousarray(w2bd),
    }


def host_gn1(accx1, accx1sq, g1, t1g, NK):
    """Host-side GN1 scale/shift from merged per-channel partial sums."""
    mean1 = accx1.sum(1, dtype=np.float64) / NK
    ex2 = accx1sq.sum(1, dtype=np.float64) / NK
    gm = mean1.reshape(8, 4).mean(1)
    gex2 = ex2.reshape(8, 4).mean(1)
    var = gex2 - gm * gm
    s = np.asarray(g1, np.float64) / np.sqrt(var.repeat(4) + EPS)
    t = np.asarray(t1g, np.float64) - s * gm.repeat(4)
    return s.astype(np.float32), t.astype(np.float32)


def unpack_ypool(packed):
    """(128, R//2) packed pool -> (64, R): block m holds row-tiles
    4*(m//2)+2*(m%2) (parts 0:64) and +1 (parts 64:128)."""
    R = packed.shape[1] * 2
    yp = np.empty((64, R), np.float32)
    for m in range(packed.shape[1] // 128):
        rt = 4 * (m // 2) + 2 * (m % 2)
        yp[:, rt * 128:(rt + 1) * 128] = packed[0:64,
                                                m * 128:(m + 1) * 128]
        yp[:, (rt + 1) * 128:(rt + 2) * 128] = packed[64:128,
                                                      m * 128:(m + 1) * 128]
    return yp


def host_gn2(acczs, accysqs, weights, NK):
    """GN2 scale/shift from the two cores' partial sums of one batch."""
    w2 = np.asarray(weights["w2d_1"], np.float64)            # (64, 32)
    b2f = (np.asarray(weights["b2d_1"], np.float64)
           - w2.sum(1))                                       # +1 fold
    zsum = np.zeros(32, np.float64)
    eysq = np.zeros(64, np.float64)
    for accz, accysq in zip(acczs, accysqs):
        zsum += accz.astype(np.float64).reshape(4, 32, -1).sum((0, 2))
        eysq += accysq.astype(np.float64).reshape(2, 64, -1).sum((0, 2))
    ymean = (w2 @ zsum) / NK
    mean_t = ymean + b2f
    e2_t = eysq / NK + 2.0 * b2f * ymean + b2f * b2f
    gm = mean_t.reshape(8, 8).mean(1)
    ge2 = e2_t.reshape(8, 8).mean(1)
    var = ge2 - gm * gm
    g2 = np.asarray(weights["g2d_1"], np.float64)
    t2 = np.asarray(weights["t2d_1"], np.float64)
    s = g2 / np.sqrt(var.repeat(8) + EPS)
    # out = g*((y+b2f) - gm)/sd + t2 = s*y + (t2 - s*gm + s*b2f)
    t = t2 - s * gm.repeat(8) + s * b2f
    return s.astype(np.float32), t.astype(np.float32)


def prep_C(ypool, s2, t2, weights, hh):
    w3T = np.asarray(weights["w1d_0"]).T.astype(np.float32)   # (64, 128)
    w3T2 = np.concatenate([w3T, w3T], 0)                      # (128, 128)
    w4T = np.asarray(weights["w1d_1"]).T.astype(np.float32)   # (128, 256)
    b3f = (np.asarray(weights["b1d_0"])
           - np.asarray(weights["w1d_0"]).sum(1))
    b4f = (np.asarray(weights["b1d_1"])
           - np.asarray(weights["w1d_1"]).sum(1))
    sl4 = slice(hh * 128, (hh + 1) * 128)
    m = {
        "ypool": np.ascontiguousarray(ypool),
        "s2t": np.ascontiguousarray(np.tile(s2, 2).reshape(-1, 1).astype(np.float32)),
        "t2t": np.ascontiguousarray(np.tile(t2, 2).reshape(-1, 1).astype(np.float32)),
        "w3T2": np.ascontiguousarray(w3T2),
        "w4h": np.ascontiguousarray(w4T[:, sl4]),
        "b3": b3f.reshape(-1, 1).astype(np.float32),
        "g3": np.asarray(weights["g1d_0"]).reshape(-1, 1).astype(np.float32),
        "t3": np.asarray(weights["t1d_0"]).reshape(-1, 1).astype(np.float32),
        "b4": np.ascontiguousarray(b4f[sl4].reshape(-1, 1).astype(np.float32)),
        "g4": np.ascontiguousarray(np.asarray(weights["g1d_1"])[sl4].reshape(-1, 1).astype(np.float32)),
        "t4": np.ascontiguousarray(np.asarray(weights["t1d_1"])[sl4].reshape(-1, 1).astype(np.float32)),
    }
    for nm, (cc, gg) in (("c128", (128, 8)), ("c128h", (128, 4))):
        indT, bindT = _inds(cc, gg)
        m[f"indT_{nm}"] = indT
        m[f"bindT_{nm}"] = bindT
    return m


_CACHE = {}


def _build_A(N, R, ch=1024):
    key = ("A", N, R, ch)
    if key in _CACHE:
        return _CACHE[key]
    nc = bacc.Bacc("TRN2", target_bir_lowering=False, debug=False,
                   num_devices=8)
    NTh = R // 128
    ins = {
        "pts": nc.dram_tensor("pts", [4, R], F32, kind="ExternalInput").ap(),
        "qb": nc.dram_tensor("qb", [21, R], BF, kind="ExternalInput").ap(),
        "cb": nc.dram_tensor("cb", [21, N], BF, kind="ExternalInput").ap(),
        "tbl": nc.dram_tensor("tbl", [N, 4], F32, kind="ExternalInput").ap(),
        "w1bTg": nc.dram_tensor("w1bTg", [68, 32 * 16], F32,
                                kind="ExternalInput").ap(),
    }
    outs = {
        "x1o": nc.dram_tensor("x1o", [NTh, 32, 128 * K], F32,
                              kind="ExternalOutput").ap(),
        "accx1o": nc.dram_tensor("accx1o", [32, NTh * 4], F32,
                                 kind="ExternalOutput").ap(),
        "accx1sqo": nc.dram_tensor("accx1sqo", [32, NTh * 4], F32,
                                   kind="ExternalOutput").ap(),
    }
    with tile.TileContext(nc) as tc:
        build_phaseA(tc, outs, ins, N, R, ch)
    nc.compile()
    _CACHE[key] = nc
    return nc


def _build_B(R):
    key = ("B", R)
    if key in _CACHE:
        return _CACHE[key]
    nc = bacc.Bacc("TRN2", target_bir_lowering=False, debug=False,
                   num_devices=8)
    NTh = R // 128
    NP = R // 512
    ins = {
        "x1s": nc.dram_tensor("x1s", [NP, 128, 128 * K], F32,
                              kind="ExternalInput").ap(),
        "s1t": nc.dram_tensor("s1t", [128, 1], F32, kind="ExternalInput").ap(),
        "t1t": nc.dram_tensor("t1t", [128, 1], F32, kind="ExternalInput").ap(),
        "w2bd": nc.dram_tensor("w2bd", [128, 128], F32,
                               kind="ExternalInput").ap(),
    }
    outs = {
        "ypoolo": nc.dram_tensor("ypoolo", [128, R // 2], F32,
                                 kind="ExternalOutput").ap(),
        "acczo": nc.dram_tensor("acczo", [128, 2 * NP], F32,
                                kind="ExternalOutput").ap(),
        "accysqo": nc.dram_tensor("accysqo", [128, 2 * NP], F32,
                                  kind="ExternalOutput").ap(),
    }
    with tile.TileContext(nc) as tc:
        build_phaseB(tc, outs, ins, R)
    nc.compile()
    _CACHE[key] = nc
    return nc


def _build_C(N):
    key = ("C", N)
    if key in _CACHE:
        return _CACHE[key]
    nc = bacc.Bacc("TRN2", target_bir_lowering=False, debug=False,
                   num_devices=8)
    ins = {"ypool": nc.dram_tensor("ypool", [64, N], F32,
                                   kind="ExternalInput").ap()}
    specs = [("s2t", [128, 1]), ("t2t", [128, 1]), ("w3T2", [128, 128]),
             ("w4h", [128, 128]), ("b3", [128, 1]), ("g3", [128, 1]),
             ("t3", [128, 1]), ("b4", [128, 1]), ("g4", [128, 1]),
             ("t4", [128, 1]),
             ("indT_c128", [128, 8]), ("bindT_c128", [8, 128]),
             ("indT_c128h", [128, 4]), ("bindT_c128h", [4, 128])]
    for nm, shape in specs:
        ins[nm] = nc.dram_tensor(nm, shape, F32, kind="ExternalInput").ap()
    outs = {"out": nc.dram_tensor("out", [128, N], F32,
                                  kind="ExternalOutput").ap()}
    with tile.TileContext(nc) as tc:
        build_phaseC(tc, outs, ins, N)
    nc.compile()
    _CACHE[key] = nc
    return nc


def kernel(points, _trace=False, **weights):
    points = np.asarray(points)
    Bn, _, N = points.shape
    R = N // 2
    ncA = _build_A(N, R)
    ncB = _build_B(R)
    ncC = _build_C(N)

    if _trace:
        import tempfile

    def _run(ncX, in_maps, prefix):
        kw = {}
        if _trace:
            kw = dict(trace=True, tmpdir=tempfile.mkdtemp(prefix=prefix))
        return run_bass_kernel_spmd(ncX, in_maps, core_ids=list(range(8)),
                                    **kw)

    in_maps_A = []
    for c in range(8):
        in_maps_A.append(prep_A(points[(c // 2) % Bn], weights, c % 2, R))
    resA = _run(ncA, in_maps_A, "trA_")

    g1 = np.asarray(weights["g2d_0"])
    t1g = np.asarray(weights["t2d_0"])
    in_maps_B = []
    for b in range(Bn):
        e, o = resA.results[2 * b], resA.results[2 * b + 1]
        acc = np.concatenate([e["accx1o"], o["accx1o"]], 1)
        accsq = np.concatenate([e["accx1sqo"], o["accx1sqo"]], 1)
        s1, t1 = host_gn1(acc, accsq, g1, t1g, N * K)
        for half in range(2):
            x1s = np.ascontiguousarray(resA.results[2 * b + half]["x1o"])
            in_maps_B.append(prep_B(x1s, s1, t1, weights))
    resB = _run(ncB, in_maps_B, "trB_")

    in_maps_C = []
    for b in range(Bn):
        e, o = resB.results[2 * b], resB.results[2 * b + 1]
        s2, t2 = host_gn2([e["acczo"], o["acczo"]],
                          [e["accysqo"], o["accysqo"]], weights, N * K)
        ypool = np.concatenate([unpack_ypool(e["ypoolo"]),
                                unpack_ypool(o["ypoolo"])], 1)
        for hh in range(2):
            in_maps_C.append(prep_C(ypool, s2, t2, weights, hh))
    resC = _run(ncC, in_maps_C, "trC_")

    out = np.stack([
        np.concatenate([resC.results[2 * b]["out"],
                        resC.results[2 * b + 1]["out"]], 0)
        for b in range(Bn)], 0)
    if _trace:
        kernel.last_exec_A = resA.exec_time_ns
        kernel.last_exec_B = resB.exec_time_ns
        kernel.last_exec_C = resC.exec_time_ns
        kernel.last_exec_ns = ((resA.exec_time_ns or 0)
                               + (resB.exec_time_ns or 0)
                               + (resC.exec_time_ns or 0))
        kernel.trace_A = resA.instructions_and_trace
        kernel.trace_B = resB.instructions_and_trace
        kernel.trace_C = resC.instructions_and_trace
    return out.astype(np.float32)
